# revision 1
# baseline (speedup 1.0000x reference)
"""DNC forward (single step) on 8 NeuronCores — Bass/Tile kernel.

Data parallel: 16 batches -> 2 per core. Key algebraic facts exploited
(valid for the prev_state==None path of the reference):

* prev_rw is uniform (1/N)  => fwd/bwd temporal read weights only need the
  row-sums and column-sums of L_new, never L_new itself.  With
  rowsum0 = L@1, Lw = L@w, colsum0 = 1@L, cw = w@L (w = write weights):
      rowsum_Lnew = (1-w)*rowsum0 - Lw + w*(sum(p) - p)
      colsum_Lnew = (1-w)*colsum0 - cw + p*(sum(w) - w)
  so L is streamed exactly once from HBM (the memory-bound roofline).
* var_phi / usage are constant across slots => argsort is the identity and
  allocation[n] = (1-u) * u^(n+1) with u = 1e-4 * prod_r(1 - free_gate_r/N).

Per 1 MB row-block of L (128 rows x 2048 cols) the four reductions run on
three different engines concurrently with the DMA stream:
  PE:  [1,w]^T @ block                       -> colsum0/cw (psum accumulate)
  DVE: tensor_tensor_reduce(block * w_bcast) -> Lw
  ACT: activation(Copy, accum_out)           -> rowsum0
All slot-indexed vectors live in a (128 partitions x 16 chunks) layout.
"""
import numpy as np
from contextlib import ExitStack

import concourse.bass as bass
import concourse.bacc as bacc
import concourse.tile as tile
from concourse import mybir
from concourse.bass_utils import run_bass_kernel_spmd

F32 = mybir.dt.float32
BF16 = mybir.dt.bfloat16
AF = mybir.ActivationFunctionType
OP = mybir.AluOpType

NCORES = 8
BC = 2                  # batches per core
N = 2048                # memory slots
NCH = N // 128          # 16 slot chunks
WD = 64                 # word size
R = 4                   # read heads
IN_D, H_D, IFACE = 256, 512, 727
EPS = 1e-8

# interface vector slice offsets
O_RK, O_RS, O_WK, O_WS = 0, 256, 260, 324
O_ER, O_WV, O_FG, O_AG, O_WG, O_RM = 325, 389, 453, 457, 458, 459


def _build_pre(nc, pools, aps, b):
    """Pre-L phase: controller, write addressing, memory update, read keys."""
    (bpool, bfat, lpool, scr_ttr, scr_act, scr64, pss, pbig, lbf, consts) = pools
    ones_row, ones_col, one_one, i128, iota, ones64, ones_row_bf = consts
    x_ap, mem_ap, l_ap, p_ap, out_ap = (
        aps['x'], aps['memory'], aps['L'], aps['p'], aps['out'])
    w1_sb, w2_sb, b1_sb, b2_sb = aps['w1_sb'], aps['w2_sb'], aps['b1_sb'], aps['b2_sb']

    act = nc.scalar
    dve = nc.vector
    gp = nc.gpsimd
    pe = nc.tensor

    def mm(out, lhsT, rhs, start=True, stop=True):
        pe.matmul(out, lhsT, rhs, start=start, stop=stop)

    def ps_small(p_, f):
        return pss.tile([p_, f], F32, tag="pss", name="pss")

    def sb(p_, f, tag):
        return bpool.tile([p_, f], F32, tag=tag, name=tag)

    def fat(p_, f, tag):
        return bfat.tile([p_, f], F32, tag=tag, name=tag)

    # -------- controller --------
    xb = sb(1, IN_D, "xb")
    nc.sync.dma_start(xb[:], x_ap[b:b + 1, :])

    xT = sb(128, 2, "xT")
    ptx = ps_small(128, 2)
    for c in range(2):
        mm(ptx[:, c:c + 1], xb[0:1, 128 * c:128 * (c + 1)], one_one[:])
    dve.tensor_copy(xT[:], ptx[:])

    h_ps = ps_small(1, H_D)
    for c in range(2):
        mm(h_ps[:], xT[:, c:c + 1], w1_sb[:, c, :], start=(c == 0), stop=(c == 1))
    h_lin = sb(1, H_D, "h_lin")
    dve.tensor_tensor(h_lin[:], h_ps[:], b1_sb[:], op=OP.add)
    h_sb = sb(1, H_D, "h_sb")
    act.activation(h_sb[:], h_lin[:], AF.Tanh)

    hT = sb(128, 4, "hT")
    pth = ps_small(128, 4)
    for c in range(4):
        mm(pth[:, c:c + 1], h_sb[0:1, 128 * c:128 * (c + 1)], one_one[:])
    dve.tensor_copy(hT[:], pth[:])

    v_sb = sb(1, IFACE, "v_sb")
    for lo, hi in ((0, 512), (512, IFACE)):
        v_ps = ps_small(1, hi - lo)
        for c in range(4):
            mm(v_ps[:], hT[:, c:c + 1], w2_sb[:, c, lo:hi],
               start=(c == 0), stop=(c == 3))
        dve.tensor_tensor(v_sb[0:1, lo:hi], v_ps[:], b2_sb[0:1, lo:hi], op=OP.add)

    # -------- interface nonlinearities --------
    er_sg = sb(1, WD, "er_sg")
    act.activation(er_sg[:], v_sb[0:1, O_ER:O_ER + WD], AF.Sigmoid)
    fg_sg = sb(1, R, "fg_sg")
    act.activation(fg_sg[:], v_sb[0:1, O_FG:O_FG + R], AF.Sigmoid)
    ag_sg = sb(1, 1, "ag_sg")
    act.activation(ag_sg[:], v_sb[0:1, O_AG:O_AG + 1], AF.Sigmoid)
    wg_sg = sb(1, 1, "wg_sg")
    act.activation(wg_sg[:], v_sb[0:1, O_WG:O_WG + 1], AF.Sigmoid)

    rs_s = sb(1, R, "rs_s")         # 1 + softplus(read strengths)
    act.activation(rs_s[:], v_sb[0:1, O_RS:O_RS + R], AF.Exp)
    act.activation(rs_s[:], rs_s[:], AF.Ln, bias=1.0)
    act.activation(rs_s[:], rs_s[:], AF.Copy, bias=1.0)
    ws_s = sb(1, 1, "ws_s")
    act.activation(ws_s[:], v_sb[0:1, O_WS:O_WS + 1], AF.Exp)
    act.activation(ws_s[:], ws_s[:], AF.Ln, bias=1.0)
    act.activation(ws_s[:], ws_s[:], AF.Copy, bias=1.0)

    rm_e = sb(1, 3 * R, "rm_e")
    act.activation(rm_e[:], v_sb[0:1, O_RM:O_RM + 3 * R], AF.Exp)
    rm_sum = sb(1, R, "rm_sum")
    dve.tensor_reduce(rm_sum[:], rm_e[:].rearrange("o (r t) -> o r t", t=3),
                      axis=mybir.AxisListType.X, op=OP.add)
    rm_rec = sb(1, R, "rm_rec")
    dve.reciprocal(rm_rec[:], rm_sum[:])
    modes = sb(1, 3 * R, "modes")
    dve.tensor_tensor(modes[:].rearrange("o (r t) -> o r t", t=3),
                      rm_e[:].rearrange("o (r t) -> o r t", t=3),
                      rm_rec[:].rearrange("o (r t) -> o r t", t=1)
                      .broadcast_to([1, R, 3]),
                      op=OP.mult)

    # -------- usage scalar u, allocation params --------
    fgN = sb(1, R, "fgN")
    act.activation(fgN[:], fg_sg[:], AF.Copy, scale=-1.0 / N, bias=1.0)
    fg2 = sb(1, 2, "fg2")
    dve.tensor_tensor(fg2[:], fgN[0:1, 0:2], fgN[0:1, 2:4], op=OP.mult)
    prod = sb(1, 1, "prod")
    dve.tensor_tensor(prod[:], fg2[0:1, 0:1], fg2[0:1, 1:2], op=OP.mult)
    u_sb = sb(1, 1, "u_sb")
    act.activation(u_sb[:], prod[:], AF.Copy, scale=1e-4)
    ln_u = sb(1, 1, "ln_u")
    act.activation(ln_u[:], u_sb[:], AF.Ln)
    omu = sb(1, 1, "omu")
    act.activation(omu[:], u_sb[:], AF.Copy, scale=-1.0, bias=1.0)

    # -------- memory load + row norms --------
    M_sb = bfat.tile([128, NCH * WD], F32, tag="M_sb", name="M_sb",
                      bufs=1)
    M3 = M_sb[:].rearrange("q (i w) -> q i w", w=WD)
    nc.sync.dma_start(M3, mem_ap[b].rearrange("(i q) w -> q i w", q=128))

    msq = sb(128, NCH, "msq")
    sq1 = scr_act.tile([128, NCH * WD], F32, tag="sact", name="sact")
    dve.tensor_tensor(sq1[:], M_sb[:], M_sb[:], op=OP.mult)
    dve.tensor_reduce(msq[:], sq1[:].rearrange(
        "q (i w) -> q i w", w=WD), axis=mybir.AxisListType.X, op=OP.add)
    mn_s = sb(128, NCH, "mn_s")
    act.activation(mn_s[:], msq[:], AF.Sqrt)
    dve.tensor_scalar_add(mn_s[:], mn_s[:], EPS)
    rn_w = sb(128, NCH, "rn_w")
    dve.reciprocal(rn_w[:], mn_s[:])

    # -------- write key normalization + content scores (gpsimd dot) --------
    wk2 = sb(1, 1, "wk2")
    s64b = scr64.tile([128, WD], F32, tag="s64", name="s64")
    act.activation(s64b[0:1, :], v_sb[0:1, O_WK:O_WK + WD], AF.Square,
                   accum_out=wk2[:])
    nk = sb(1, 1, "nk")
    act.activation(nk[:], wk2[:], AF.Sqrt)
    snk = sb(1, 1, "snk")
    dve.tensor_tensor(snk[:], ws_s[:], nk[:], op=OP.mult)
    act.activation(snk[:], snk[:], AF.Copy, bias=EPS)
    srec = sb(1, 1, "srec")
    dve.reciprocal(srec[:], snk[:])
    wf = sb(1, 1, "wf")
    dve.tensor_tensor(wf[:], ws_s[:], srec[:], op=OP.mult)
    kn = sb(1, WD, "kn")
    act.activation(kn[:], v_sb[0:1, O_WK:O_WK + WD], AF.Copy, scale=wf[:])
    kn_bc = sb(128, WD, "kn_bc")
    pt = ps_small(128, WD)
    mm(pt[:], ones_row[:], kn[:])
    dve.tensor_copy(kn_bc[:], pt[:])

    wsc_r = sb(128, NCH, "wsc_r")   # raw dot(M_n, kn) per slot
    for i in range(NCH):
        g64 = scr64.tile([128, WD], F32, tag="g64", name="g64")
        dve.scalar_tensor_tensor(out=g64[:], in0=M3[:, i, :], scalar=1.0,
                                 in1=kn_bc[:], op0=OP.mult, op1=OP.mult,
                                 accum_out=wsc_r[:, i:i + 1])
    wsc = sb(128, NCH, "wsc")
    dve.tensor_tensor(wsc[:], wsc_r[:], rn_w[:], op=OP.mult)

    # softmax over all 2048 slots
    wse = sb(128, NCH, "wse")
    wse_s = sb(128, 1, "wse_s")
    act.activation(wse[:], wsc[:], AF.Exp, accum_out=wse_s[:])
    ptt = ps_small(1, 1)
    mm(ptt[:], wse_s[:], ones_col[:])
    totr = sb(1, 1, "totr")
    dve.reciprocal(totr[:], ptt[:])

    # batch the per-batch scalars into one broadcast matmul:
    # [ln_u, 1-u, c1=wg*ag, c2=wg*(1-ag), 1/sum(exp(wsc))]
    omag = sb(1, 1, "omag")
    act.activation(omag[:], ag_sg[:], AF.Copy, scale=-1.0, bias=1.0)
    c1 = sb(1, 1, "c1")
    dve.tensor_tensor(c1[:], wg_sg[:], ag_sg[:], op=OP.mult)
    c2 = sb(1, 1, "c2")
    dve.tensor_tensor(c2[:], wg_sg[:], omag[:], op=OP.mult)
    sc5 = sb(1, 5, "sc5")
    for j, t in enumerate((ln_u, omu, c1, c2, totr)):
        dve.tensor_copy(sc5[0:1, j:j + 1], t[:])
    pb5 = ps_small(128, 5)
    mm(pb5[:], ones_row[:], sc5[:])
    scb = sb(128, 5, "scb")
    dve.tensor_copy(scb[:], pb5[:])

    # allocation = (1-u) * u^(n+1) and write weights
    alle = sb(128, NCH, "alle")
    act.activation(alle[:], iota[:], AF.Exp, scale=scb[:, 0:1])
    alloc = sb(128, NCH, "alloc")
    act.activation(alloc[:], alle[:], AF.Copy, scale=scb[:, 1:2])

    cww = sb(128, NCH, "cww")
    dve.tensor_scalar_mul(cww[:], wse[:], scb[:, 4:5])
    t2 = sb(128, NCH, "t2w")
    dve.tensor_scalar_mul(t2[:], cww[:], scb[:, 3:4])
    w_sb = sb(128, NCH, "w_sb")
    dve.scalar_tensor_tensor(out=w_sb[:], in0=alloc[:], scalar=scb[:, 2:3],
                             in1=t2[:], op0=OP.mult, op1=OP.add)

    # -------- w-derived operands for the L pass --------
    oww = bpool.tile([128, 2 * NCH], BF16, tag="oww", name="oww")
    oww3 = oww[:].rearrange("q (t i) -> q t i", i=NCH)
    dve.memset(oww3[:, 0, :], 1.0)
    dve.tensor_copy(oww3[:, 1, :], w_sb[:])

    wrow = bfat.tile([1, N], F32, tag="wrow", name="wrow", bufs=1)
    wrow_bf = bpool.tile([1, N], BF16, tag="wrow_bf", name="wrow_bf")
    w_bc = bfat.tile([128, N], BF16, tag="w_bc", name="w_bc")
    for g in range(4):
        pr = ps_small(1, 512)
        for j in range(4):
            c = 4 * g + j
            mm(pr[0:1, 128 * j:128 * (j + 1)], w_sb[:, c:c + 1], i128[:])
        dve.tensor_copy(wrow[0:1, 512 * g:512 * (g + 1)], pr[:])
        dve.tensor_copy(wrow_bf[0:1, 512 * g:512 * (g + 1)], pr[:])
        pb = ps_small(128, 512)
        mm(pb[:], ones_row_bf[:], wrow_bf[0:1, 512 * g:512 * (g + 1)])
        act.copy(w_bc[:, 512 * g:512 * (g + 1)], pb[:])

    wsum = sb(1, 1, "wsum")
    pws = ps_small(1, NCH)
    mm(pws[:], ones_col[:], w_sb[:])
    ws16 = sb(1, NCH, "ws16")
    dve.tensor_copy(ws16[:], pws[:])
    dve.tensor_reduce(wsum[:], ws16[:], axis=mybir.AxisListType.X, op=OP.add)

    psum_s = sb(1, 1, "psum_s")
    pT = sb(128, NCH, "pT")
    nc.sync.dma_start(
        pT[:].rearrange("q (c o) -> q c o", o=1),
        p_ap[b, 0:1, :].rearrange("o (c q) -> q c o", q=128))
    pps = ps_small(1, NCH)
    mm(pps[:], ones_col[:], pT[:])
    ps16 = sb(1, NCH, "ps16")
    dve.tensor_copy(ps16[:], pps[:])
    dve.tensor_reduce(psum_s[:], ps16[:], axis=mybir.AxisListType.X, op=OP.add)

    pw2 = sb(1, 2, "pw2")
    dve.tensor_copy(pw2[0:1, 0:1], psum_s[:])
    dve.tensor_copy(pw2[0:1, 1:2], wsum[:])
    pbx = ps_small(128, 2)
    mm(pbx[:], ones_row[:], pw2[:])
    pwb = sb(128, 2, "pwb")
    dve.tensor_copy(pwb[:], pbx[:])

    # -------- memory update (independent of L; overlaps the stream) --------
    # M_new = M * F + G with rank-1 F = 1 - w (x) e, G = w (x) v built on PE
    ev = bpool.tile([1, 2 * WD], F32, tag="ev", name="ev")
    dve.tensor_copy(ev[0:1, 0:WD], er_sg[:])
    dve.tensor_copy(ev[0:1, WD:2 * WD], v_sb[0:1, O_WV:O_WV + WD])
    FG = bfat.tile([128, NCH * 2 * WD], F32, tag="FG", name="FG",
                    bufs=1)
    FG3 = FG[:].rearrange("q (i w) -> q i w", w=2 * WD)
    for i in range(NCH):
        pt = ps_small(128, 2 * WD)
        mm(pt[:], wrow[0:1, 128 * i:128 * (i + 1)], ev[:])
        dve.scalar_tensor_tensor(out=FG3[:, i, 0:WD], in0=pt[:, 0:WD],
                                 scalar=-1.0, in1=ones64[:, 0:WD],
                                 op0=OP.mult, op1=OP.add)
        dve.tensor_copy(FG3[:, i, WD:2 * WD], pt[:, WD:2 * WD])

    Mn_sb = fat(128, NCH * WD, "Mn_sb")
    Mn3 = Mn_sb[:].rearrange("q (i w) -> q i w", w=WD)
    for i in range(NCH):
        g1 = scr64.tile([128, WD], F32, tag="g64", name="g64")
        gp.tensor_tensor(g1[:], M3[:, i, :], FG3[:, i, 0:WD], op=OP.mult)
        gp.tensor_tensor(Mn3[:, i, :], g1[:], FG3[:, i, WD:2 * WD], op=OP.add)

    mq2 = sb(128, NCH, "mq2")
    sq2 = scr_act.tile([128, NCH * WD], F32, tag="sact", name="sact")
    dve.tensor_tensor(sq2[:], Mn_sb[:], Mn_sb[:], op=OP.mult)
    dve.tensor_reduce(mq2[:], sq2[:].rearrange(
        "q (i w) -> q i w", w=WD), axis=mybir.AxisListType.X, op=OP.add)
    mn2 = sb(128, NCH, "mn2")
    act.activation(mn2[:], mq2[:], AF.Sqrt)
    dve.tensor_scalar_add(mn2[:], mn2[:], EPS)
    rn2 = sb(128, NCH, "rn2")
    dve.reciprocal(rn2[:], mn2[:])

    MnT = bfat.tile([64, NCH * 128], F32, tag="MnT", name="MnT",
                     bufs=1)
    MnT3 = MnT[:].rearrange("q (i c) -> q i c", c=128)
    for g in range(4):
        pt = ps_small(64, 512)
        for j in range(4):
            pe.transpose(pt[:, 128 * j:128 * (j + 1)], Mn3[:, 4 * g + j, :],
                         i128[:])
        act.copy(MnT[0:64, 512 * g:512 * (g + 1)], pt[:])

    # -------- read keys --------
    rk2 = sb(1, R, "rk2")
    for r in range(R):
        s64 = scr64.tile([128, WD], F32, tag="s64", name="s64")
        act.activation(s64[0:1, :], v_sb[0:1, O_RK + WD * r:O_RK + WD * (r + 1)],
                       AF.Square, accum_out=rk2[0:1, r:r + 1])
    rkn_n = sb(1, R, "rkn_n")
    act.activation(rkn_n[:], rk2[:], AF.Sqrt)
    srn = sb(1, R, "srn")
    dve.tensor_tensor(srn[:], rs_s[:], rkn_n[:], op=OP.mult)
    act.activation(srn[:], srn[:], AF.Copy, bias=EPS)
    rrec = sb(1, R, "rrec")
    dve.reciprocal(rrec[:], srn[:])
    rf = sb(1, R, "rf")
    dve.tensor_tensor(rf[:], rs_s[:], rrec[:], op=OP.mult)
    rkn = sb(1, R * WD, "rkn")
    dve.tensor_tensor(rkn[:].rearrange("o (r w) -> o r w", w=WD),
                      v_sb[0:1, O_RK:O_RK + R * WD]
                      .rearrange("o (r w) -> o r w", w=WD),
                      rf[:].rearrange("o (r w) -> o r w", w=1)
                      .broadcast_to([1, R, WD]),
                      op=OP.mult)
    rknT = sb(64, R, "rknT")
    ptk = ps_small(64, R)
    for r in range(R):
        mm(ptk[:, r:r + 1], rkn[0:1, WD * r:WD * (r + 1)], one_one[:])
    dve.tensor_copy(rknT[:], ptk[:])

    # -------- read content scores + per-head softmax pieces --------
    rsc = sb(128, R * NCH, "rsc")
    rsc3 = rsc[:].rearrange("q (r i) -> q r i", i=NCH)
    for i in range(NCH):
        pt = ps_small(128, R)
        mm(pt[:], MnT3[:, i, :], rknT[:])
        dve.tensor_scalar_mul(rsc3[:, :, i], pt[:], rn2[:, i:i + 1])
    rex = sb(128, R * NCH, "rex")
    rex3 = rex[:].rearrange("q (r i) -> q r i", i=NCH)
    res_s = sb(128, R, "res_s")
    for r in range(R):
        act.activation(rex3[:, r, :], rsc3[:, r, :], AF.Exp,
                       accum_out=res_s[:, r:r + 1])
    ptot = ps_small(R, 1)
    mm(ptot[:], res_s[:], ones_col[:])
    rec4 = sb(R, 1, "rec4")
    dve.reciprocal(rec4[:], ptot[:])
    prr = ps_small(1, R)
    mm(prr[:], rec4[:], i128[0:R, 0:R])
    rec_row = sb(1, R, "rec_row")
    dve.tensor_copy(rec_row[:], prr[:])

    return dict(oww3=oww3, w_bc=w_bc, pT=pT, pwb=pwb, w_sb=w_sb,
                modes=modes, rec_row=rec_row, rex3=rex3, Mn3=Mn3)


def _build_post(nc, pools, aps, b, st):
    """L streaming pass + temporal weights + read vectors."""
    (bpool, bfat, lpool, scr_ttr, scr_act, scr64, pss, pbig, lbf, consts) = pools
    ones_row, ones_col, one_one, i128, iota, ones64, ones_row_bf = consts
    l_ap, out_ap = aps['L'], aps['out']
    act = nc.scalar
    dve = nc.vector
    gp = nc.gpsimd
    pe = nc.tensor

    def mm(out, lhsT, rhs, start=True, stop=True):
        pe.matmul(out, lhsT, rhs, start=start, stop=stop)

    def ps_small(p_, f):
        return pss.tile([p_, f], F32, tag="pss", name="pss")

    def sb(p_, f, tag):
        return bpool.tile([p_, f], F32, tag=tag, name=tag)

    def fat(p_, f, tag):
        return bfat.tile([p_, f], F32, tag=tag, name=tag)

    oww3, w_bc, pT, pwb, w_sb = (st['oww3'], st['w_bc'], st['pT'], st['pwb'],
                                 st['w_sb'])
    modes, rec_row, rex3, Mn3 = (st['modes'], st['rec_row'], st['rex3'],
                                 st['Mn3'])

    # -------- the L pass: stream 16 row blocks of 1 MB --------
    cscw_ps = pbig.tile([2, N], F32, tag="cscw", name="cscw")
    rs0 = sb(128, NCH, "rs0")
    lw = sb(128, NCH, "lw")
    for i in range(NCH):
        lblk = lpool.tile([128, N], F32, tag="lblk", name="lblk")
        nc.sync.dma_start(lblk[:], l_ap[b, 128 * i:128 * (i + 1), :])
        lb = lbf.tile([128, N], BF16, tag="lbf", name="lbf")
        act.activation(lb[:], lblk[:], AF.Copy, accum_out=rs0[:, i:i + 1])
        for c in range(4):
            mm(cscw_ps[:, 512 * c:512 * (c + 1)], oww3[:, :, i],
               lb[:, 512 * c:512 * (c + 1)],
               start=(i == 0), stop=(i == NCH - 1))
        sT = scr_ttr.tile([128, N], BF16, tag="sttr", name="sttr")
        dve.scalar_tensor_tensor(out=sT[:], in0=lb[:], scalar=1.0,
                                 in1=w_bc[:], op0=OP.mult, op1=OP.mult,
                                 accum_out=lw[:, i:i + 1])

    # -------- temporal weights from the four L sums --------
    cscw_sb = bfat.tile([2, N], F32, tag="cscw_sb", name="cscw_sb",
                         bufs=1)
    act.copy(cscw_sb[:], cscw_ps[:])
    csT = sb(128, 2 * NCH, "csT")
    csT3 = csT[:].rearrange("q (i t) -> q i t", t=2)
    ptc = ps_small(128, 2 * NCH)
    for c in range(NCH):
        mm(ptc[:, 2 * c:2 * c + 2], cscw_sb[0:2, 128 * c:128 * (c + 1)],
           i128[0:2, 0:2])
    dve.tensor_copy(csT[:], ptc[:])
    cs0T = csT3[:, :, 0]
    cwT = csT3[:, :, 1]

    # rowsum_Lnew = rs0 - w*rs0 - Lw + w*(P_sum - p)
    pwb0 = pwb[:, 0:1].rearrange("q (a o) -> q a o", a=1).broadcast_to(
        [128, 1, NCH])[:, 0, :]
    r_t1 = sb(128, NCH, "r_t1")
    gp.tensor_tensor(r_t1[:], pwb0, pT[:], op=OP.subtract)
    r_t2 = sb(128, NCH, "r_t2")
    gp.tensor_tensor(r_t2[:], w_sb[:], r_t1[:], op=OP.mult)
    r_u1 = sb(128, NCH, "r_u1")
    gp.tensor_tensor(r_u1[:], w_sb[:], rs0[:], op=OP.mult)
    r_s1 = sb(128, NCH, "r_s1")
    gp.tensor_tensor(r_s1[:], rs0[:], r_u1[:], op=OP.subtract)
    r_s2 = sb(128, NCH, "r_s2")
    gp.tensor_tensor(r_s2[:], r_s1[:], lw[:], op=OP.subtract)
    rrow_f = sb(128, NCH, "rrow_f")
    gp.tensor_tensor(rrow_f[:], r_s2[:], r_t2[:], op=OP.add)
    ebw = sb(128, NCH, "ebw")
    ebw_s = sb(128, 1, "ebw_s")
    act.activation(ebw[:], rrow_f[:], AF.Exp, scale=1.0 / N, accum_out=ebw_s[:])

    # colsum_Lnew = cs0 - w*cs0 - cw + p*(W_sum - w)
    pwb1 = pwb[:, 1:2].rearrange("q (a o) -> q a o", a=1).broadcast_to(
        [128, 1, NCH])[:, 0, :]
    c_t1 = sb(128, NCH, "c_t1")
    gp.tensor_tensor(c_t1[:], pwb1, w_sb[:], op=OP.subtract)
    c_t2 = sb(128, NCH, "c_t2")
    gp.tensor_tensor(c_t2[:], pT[:], c_t1[:], op=OP.mult)
    c_u1 = sb(128, NCH, "c_u1")
    gp.tensor_tensor(c_u1[:], w_sb[:], cs0T, op=OP.mult)
    c_s1 = sb(128, NCH, "c_s1")
    gp.tensor_tensor(c_s1[:], cs0T, c_u1[:], op=OP.subtract)
    c_s2 = sb(128, NCH, "c_s2")
    gp.tensor_tensor(c_s2[:], c_s1[:], cwT, op=OP.subtract)
    crow_f = sb(128, NCH, "crow_f")
    gp.tensor_tensor(crow_f[:], c_s2[:], c_t2[:], op=OP.add)
    efw = sb(128, NCH, "efw")
    efw_s = sb(128, 1, "efw_s")
    act.activation(efw[:], crow_f[:], AF.Exp, scale=1.0 / N, accum_out=efw_s[:])

    pt = ps_small(1, 1)
    mm(pt[:], ebw_s[:], ones_col[:])
    rec_b = sb(1, 1, "rec_b")
    dve.reciprocal(rec_b[:], pt[:])
    pt = ps_small(1, 1)
    mm(pt[:], efw_s[:], ones_col[:])
    rec_f = sb(1, 1, "rec_f")
    dve.reciprocal(rec_f[:], pt[:])

    # per-head combine coefficients: b0 = modes[r,0]/Zbwd, b1 = modes[r,1]/Zc_r,
    # b2 = modes[r,2]/Zfwd  (softmax normalizers folded into the mode weights)
    bvec = sb(1, 3 * R, "bvec")
    dve.tensor_tensor(bvec[0:1, 0:R],
                      modes[:].rearrange("o (r t) -> o r t", t=3)[:, :, 0],
                      rec_b[0:1, 0:1].broadcast_to([1, R]), op=OP.mult)
    dve.tensor_tensor(bvec[0:1, R:2 * R],
                      modes[:].rearrange("o (r t) -> o r t", t=3)[:, :, 1],
                      rec_row[:], op=OP.mult)
    dve.tensor_tensor(bvec[0:1, 2 * R:3 * R],
                      modes[:].rearrange("o (r t) -> o r t", t=3)[:, :, 2],
                      rec_f[0:1, 0:1].broadcast_to([1, R]), op=OP.mult)
    pbv = ps_small(128, 3 * R)
    mm(pbv[:], ones_row[:], bvec[:])
    Bco = sb(128, 3 * R, "Bco")
    dve.tensor_copy(Bco[:], pbv[:])

    # read weights and read vectors
    rw_sb = sb(128, R * NCH, "rw_sb")
    rw3 = rw_sb[:].rearrange("q (r i) -> q r i", i=NCH)
    def bcast_col(col):
        return col.rearrange("q (a o) -> q a o", a=1).broadcast_to(
            [128, 1, NCH])[:, 0, :]

    for r in range(R):
        z3 = sb(128, NCH, "z3")
        act.activation(z3[:], efw[:], AF.Copy, scale=Bco[:, 2 * R + r:2 * R + r + 1])
        z2 = sb(128, NCH, "z2")
        gp.tensor_tensor(z2[:], rex3[:, r, :], bcast_col(Bco[:, R + r:R + r + 1]),
                         op=OP.mult)
        gp.tensor_tensor(z2[:], z2[:], z3[:], op=OP.add)
        gp.tensor_tensor(rw3[:, r, :], ebw[:], bcast_col(Bco[:, r:r + 1]),
                         op=OP.mult)
        gp.tensor_tensor(rw3[:, r, :], rw3[:, r, :], z2[:], op=OP.add)

    prv = pbig.tile([R, WD], F32, tag="prv", name="prv")
    rw_by_i = rw_sb[:].rearrange("q (r i) -> q i r", i=NCH)
    for i in range(NCH):
        mm(prv[:], rw_by_i[:, i, :], Mn3[:, i, :],
           start=(i == 0), stop=(i == NCH - 1))
    out_sb = sb(R, WD, "out_sb")
    dve.tensor_copy(out_sb[:], prv[:])
    nc.sync.dma_start(out_ap[b], out_sb[:])


def build_nc():
    nc = bacc.Bacc("TRN2", target_bir_lowering=False, debug=False)

    dr = {}
    dr['x'] = nc.dram_tensor("x", [BC, IN_D], F32, kind="ExternalInput").ap()
    dr['memory'] = nc.dram_tensor("memory", [BC, N, WD], F32,
                                  kind="ExternalInput").ap()
    dr['L'] = nc.dram_tensor("L", [BC, N, N], F32, kind="ExternalInput").ap()
    dr['p'] = nc.dram_tensor("p", [BC, 1, N], F32, kind="ExternalInput").ap()
    w1_ap = nc.dram_tensor("W1", [IN_D, H_D], F32, kind="ExternalInput").ap()
    b1_ap = nc.dram_tensor("b1", [1, H_D], F32, kind="ExternalInput").ap()
    w2_ap = nc.dram_tensor("W2", [H_D, IFACE], F32, kind="ExternalInput").ap()
    b2_ap = nc.dram_tensor("b2", [1, IFACE], F32, kind="ExternalInput").ap()
    iota_ap = nc.dram_tensor("iota_p1", [128, NCH], F32,
                             kind="ExternalInput").ap()
    i128_ap = nc.dram_tensor("i128", [128, 128], F32, kind="ExternalInput").ap()
    dr['out'] = nc.dram_tensor("out", [BC, R, WD], F32,
                               kind="ExternalOutput").ap()

    with tile.TileContext(nc) as tc, ExitStack() as ctx:
        persist = ctx.enter_context(tc.tile_pool(name="persist", bufs=1))
        bpool = ctx.enter_context(tc.tile_pool(name="bpool", bufs=2))
        bfat = ctx.enter_context(tc.tile_pool(name="bfat", bufs=2))
        lpool = ctx.enter_context(tc.tile_pool(name="lpool", bufs=3))
        scr_ttr = ctx.enter_context(tc.tile_pool(name="scr_ttr", bufs=1))
        scr_act = ctx.enter_context(tc.tile_pool(name="scr_act", bufs=1))
        lbf = ctx.enter_context(tc.tile_pool(name="lbf", bufs=10))
        scr64 = ctx.enter_context(tc.tile_pool(name="scr64", bufs=3))
        pss = ctx.enter_context(tc.tile_pool(name="pss", bufs=3, space="PSUM"))
        pbig = ctx.enter_context(tc.tile_pool(name="pbig", bufs=1,
                                              space="PSUM"))

        ones_row = persist.tile([1, 128], F32, tag="ones_row")
        nc.vector.memset(ones_row[:], 1.0)
        ones_col = persist.tile([128, 1], F32, tag="ones_col")
        nc.vector.memset(ones_col[:], 1.0)
        one_one = persist.tile([1, 1], F32, tag="one_one")
        nc.vector.memset(one_one[:], 1.0)
        i128 = persist.tile([128, 128], F32, tag="i128")
        nc.sync.dma_start(i128[:], i128_ap)
        iota = persist.tile([128, NCH], F32, tag="iota")
        nc.sync.dma_start(iota[:], iota_ap)
        ones64 = persist.tile([128, 2 * WD], F32, tag="ones64")
        nc.vector.memset(ones64[:], 1.0)
        ones_row_bf = persist.tile([1, 128], BF16, tag="ones_row_bf")
        nc.vector.memset(ones_row_bf[:], 1.0)

        w1_sb = persist.tile([128, 2, H_D], F32, tag="w1_sb")
        for c in range(2):
            nc.sync.dma_start(w1_sb[:, c, :], w1_ap[128 * c:128 * (c + 1), :])
        w2_sb = persist.tile([128, 4, IFACE], F32, tag="w2_sb")
        for c in range(4):
            nc.sync.dma_start(w2_sb[:, c, :], w2_ap[128 * c:128 * (c + 1), :])
        b1_sb = persist.tile([1, H_D], F32, tag="b1_sb")
        nc.sync.dma_start(b1_sb[:], b1_ap)
        b2_sb = persist.tile([1, IFACE], F32, tag="b2_sb")
        nc.sync.dma_start(b2_sb[:], b2_ap)

        aps = dict(dr)
        aps.update(w1_sb=w1_sb, w2_sb=w2_sb, b1_sb=b1_sb, b2_sb=b2_sb)
        pools = (bpool, bfat, lpool, scr_ttr, scr_act, scr64, pss, pbig, lbf,
                 (ones_row, ones_col, one_one, i128, iota, ones64,
                  ones_row_bf))
        sts = [_build_pre(nc, pools, aps, b) for b in range(BC)]
        for b in range(BC):
            _build_post(nc, pools, aps, b, sts[b])

    nc.compile()
    return nc


_NC_CACHE = []


def kernel(x, memory, L, p, W1, b1, W2, b2):
    x = np.ascontiguousarray(x, np.float32)
    memory = np.ascontiguousarray(memory, np.float32)
    L = np.ascontiguousarray(L, np.float32)
    p = np.ascontiguousarray(p, np.float32)
    W1 = np.ascontiguousarray(W1, np.float32)
    b1 = np.ascontiguousarray(b1, np.float32).reshape(1, H_D)
    W2 = np.ascontiguousarray(W2, np.float32)
    b2 = np.ascontiguousarray(b2, np.float32).reshape(1, IFACE)

    iota = (np.arange(N, dtype=np.float32).reshape(NCH, 128).T + 1.0).copy()
    i128 = np.eye(128, dtype=np.float32)

    if not _NC_CACHE:
        _NC_CACHE.append(build_nc())
    nc = _NC_CACHE[0]

    in_maps = []
    for c in range(NCORES):
        s = slice(BC * c, BC * (c + 1))
        in_maps.append({
            'x': x[s], 'memory': memory[s], 'L': L[s], 'p': p[s],
            'W1': W1, 'b1': b1, 'W2': W2, 'b2': b2,
            'iota_p1': iota, 'i128': i128,
        })

    res = run_bass_kernel_spmd(nc, in_maps, list(range(NCORES)))
    outs = [res.results[c]['out'].reshape(BC, 1, R * WD)
            for c in range(NCORES)]
    return np.concatenate(outs, axis=0)



# revision 15
# speedup vs baseline: 1.2034x; 1.2034x over previous
"""DNC forward (single step) on 8 NeuronCores — Bass/Tile kernel.

Data parallel: 16 batches -> 2 per core. Algebraic facts exploited (valid
for the prev_state==None path of the reference):

* prev_rw is uniform (1/N)  => fwd/bwd temporal read weights only need the
  row-sums and column-sums of L_new, never L_new itself.  With
  rowsum0 = L@1, Lw = L@w, colsum0 = 1@L, cw = w@L (w = write weights):
      rowsum_Lnew = (1-w)*rowsum0 - Lw + w*(sum(p) - p)
      colsum_Lnew = (1-w)*colsum0 - cw + p*(sum(w) - w)
  so L is streamed exactly once from HBM (the memory-bound roofline).
* var_phi / usage are constant across slots => argsort is the identity and
  allocation[n] = (1-u) * u^(n+1) with u = 1e-4 * prod_r(1 - free_gate_r/N).
* read/write strengths cancel inside the cosine normalization (mod the 1e-8
  eps guard), so the softplus chains are dropped.
* 1/(sqrt(x)+eps) -> exp(-0.5*ln(x)): keeps the ACT engine on a single
  exp/ln function table for the whole kernel (2 table loads total).

Schedule: the two batches' L streams are interleaved block-by-block so the
DMA queue never drains and each engine alternates batches in program order.
Per 1 MB row-block of L (128 rows x 2048 cols), three engines consume the
stream concurrently:
  ACT: Copy f32->bf16 (feeds PE) + accum    -> rowsum0 chunk
  PE:  [1,w]^T @ bf16 block (psum rows 2b)  -> colsum0/cw accumulate
  DVE: tensor_tensor_reduce(f32 blk * w_bc) -> Lw chunk
All slot-indexed vectors live in a (128 partitions x 16 chunks) layout,
slot n = 128*i + q.
"""
import numpy as np
from contextlib import ExitStack

import concourse.bass as bass
import concourse.bacc as bacc
import concourse.tile as tile
from concourse import mybir
from concourse.bass_utils import run_bass_kernel_spmd

F32 = mybir.dt.float32
BF16 = mybir.dt.bfloat16
AF = mybir.ActivationFunctionType
OP = mybir.AluOpType

NCORES = 8
BC = 2                  # batches per core
N = 2048                # memory slots
NCH = N // 128          # 16 slot chunks
WD = 64                 # word size
R = 4                   # read heads
IN_D, H_D, IFACE = 256, 512, 727
V_USED = 471            # interface cols actually used (output_vector is dead)
EPS = 1e-8

# interface vector slice offsets
O_RK, O_RS, O_WK, O_WS = 0, 256, 260, 324
O_ER, O_WV, O_FG, O_AG, O_WG, O_RM = 325, 389, 453, 457, 458, 459


def build_nc():
    nc = bacc.Bacc("TRN2", target_bir_lowering=False, debug=False)

    x_ap = nc.dram_tensor("x", [BC, IN_D], F32, kind="ExternalInput").ap()
    mem_ap = nc.dram_tensor("memory", [BC, N, WD], F32,
                            kind="ExternalInput").ap()
    l_ap = nc.dram_tensor("L", [BC, N, N], F32, kind="ExternalInput").ap()
    p_ap = nc.dram_tensor("p", [BC, 1, N], F32, kind="ExternalInput").ap()
    w1_ap = nc.dram_tensor("W1", [IN_D, H_D], F32, kind="ExternalInput").ap()
    b1_ap = nc.dram_tensor("b1", [1, H_D], F32, kind="ExternalInput").ap()
    w2_ap = nc.dram_tensor("W2", [H_D, IFACE], F32, kind="ExternalInput").ap()
    b2_ap = nc.dram_tensor("b2", [1, IFACE], F32, kind="ExternalInput").ap()
    iota_ap = nc.dram_tensor("iota_p1", [128, NCH], F32,
                             kind="ExternalInput").ap()
    i128_ap = nc.dram_tensor("i128", [128, 128], F32, kind="ExternalInput").ap()
    sel2_ap = nc.dram_tensor("sel2", [BC, BC * 128], F32,
                             kind="ExternalInput").ap()
    out_ap = nc.dram_tensor("out", [BC, R, WD], F32,
                            kind="ExternalOutput").ap()

    with tile.TileContext(nc) as tc, ExitStack() as ctx:
        act = nc.scalar
        dve = nc.vector
        gp = nc.gpsimd
        pe = nc.tensor

        persist = ctx.enter_context(tc.tile_pool(name="persist", bufs=1))
        bpool = ctx.enter_context(tc.tile_pool(name="bpool", bufs=2))
        bfat = ctx.enter_context(tc.tile_pool(name="bfat", bufs=2))
        lpool = ctx.enter_context(tc.tile_pool(name="lpool", bufs=6))
        lbf = ctx.enter_context(tc.tile_pool(name="lbf", bufs=4))
        scr = ctx.enter_context(tc.tile_pool(name="scr", bufs=1))
        bone = ctx.enter_context(tc.tile_pool(name="bone", bufs=1))
        sqp = ctx.enter_context(tc.tile_pool(name="sqp", bufs=2))
        pss = ctx.enter_context(tc.tile_pool(name="pss", bufs=2, space="PSUM"))
        pacc = ctx.enter_context(tc.tile_pool(name="pacc", bufs=2,
                                              space="PSUM"))
        pbig = ctx.enter_context(tc.tile_pool(name="pbig", bufs=1,
                                              space="PSUM"))

        def mm(out, lhsT, rhs, start=True, stop=True):
            pe.matmul(out, lhsT, rhs, start=start, stop=stop)

        def ps_small(p_, f):
            return pss.tile([p_, f], F32, tag="pss", name="pss")

        def sb(p_, f, tag):
            return bpool.tile([p_, f], F32, tag=tag, name=tag)

        # ---- constants + weights (DMA order = transfer order) ----
        ones_row = persist.tile([1, 128], F32, tag="ones_row")
        dve.memset(ones_row[:], 1.0)
        ones_col = persist.tile([128, 1], F32, tag="ones_col")
        dve.memset(ones_col[:], 1.0)
        ones_1x2 = persist.tile([1, 2], F32, tag="ones_1x2")
        dve.memset(ones_1x2[:], 1.0)
        ones_row_bf = persist.tile([1, 128], BF16, tag="ones_row_bf")
        dve.memset(ones_row_bf[:], 1.0)
        ones256 = persist.tile([128, 256], F32, tag="ones256")
        dve.memset(ones256[:], 1.0)

        xb = persist.tile([BC, IN_D], F32, tag="xb")
        nc.sync.dma_start(xb[:], x_ap[:, :])
        w1_sb = persist.tile([128, 2, H_D], F32, tag="w1_sb")
        for c in range(2):
            nc.sync.dma_start(w1_sb[:, c, :], w1_ap[128 * c:128 * (c + 1), :])
        b1_sb = persist.tile([1, H_D], F32, tag="b1_sb")
        nc.sync.dma_start(b1_sb[:], b1_ap)
        b2_sb = persist.tile([1, V_USED], F32, tag="b2_sb")
        nc.sync.dma_start(b2_sb[:], b2_ap[0:1, 0:V_USED])
        i128 = persist.tile([128, 128], F32, tag="i128")
        nc.sync.dma_start(i128[:], i128_ap)
        iota = persist.tile([128, NCH], F32, tag="iota")
        nc.sync.dma_start(iota[:], iota_ap)
        sel2 = persist.tile([BC, BC * 128], F32, tag="sel2")
        nc.sync.dma_start(sel2[:], sel2_ap)
        w2_sb = persist.tile([128, 4, V_USED], F32, tag="w2_sb")
        for c in range(4):
            nc.sync.dma_start(w2_sb[:, c, :],
                              w2_ap[128 * c:128 * (c + 1), 0:V_USED])

        M_sb, pT = [], []
        for b in range(BC):
            Mb = bone.tile([128, NCH * WD], F32, tag=f"M_sb{b}", name="M_sb")
            nc.sync.dma_start(Mb[:].rearrange("q (i w) -> q i w", w=WD),
                              mem_ap[b].rearrange("(i q) w -> q i w", q=128))
            M_sb.append(Mb)
            pb = bpool.tile([128, NCH], F32, tag="pT", name="pT")
            nc.sync.dma_start(
                pb[:].rearrange("q (c o) -> q c o", o=1),
                p_ap[b, 0:1, :].rearrange("o (c q) -> q c o", q=128))
            pT.append(pb)

        # =========== batched controller (both batches at once) ===========
        xT = sb(128, 2 * BC, "xT")          # [128, c, b]
        xT3 = xT[:].rearrange("q (c b) -> q c b", b=BC)
        ptx = ps_small(128, 2 * BC)
        for c in range(2):
            pe.transpose(ptx[:, BC * c:BC * (c + 1)],
                         xb[0:BC, 128 * c:128 * (c + 1)], i128[0:BC, 0:BC])
        dve.tensor_copy(xT[:], ptx[:])

        h_ps = ps_small(BC, H_D)
        for c in range(2):
            mm(h_ps[:], xT3[:, c, :], w1_sb[:, c, :],
               start=(c == 0), stop=False)
        mm(h_ps[:], ones_1x2[:], b1_sb[:], start=False, stop=True)
        h_sb = sb(BC, H_D, "h_sb")
        act.activation(h_sb[:], h_ps[:], AF.Tanh)

        hT = sb(128, 4 * BC, "hT")
        hT3 = hT[:].rearrange("q (c b) -> q c b", b=BC)
        pth = ps_small(128, 4 * BC)
        for c in range(4):
            pe.transpose(pth[:, BC * c:BC * (c + 1)],
                         h_sb[0:BC, 128 * c:128 * (c + 1)], i128[0:BC, 0:BC])
        dve.tensor_copy(hT[:], pth[:])

        v_ps = ps_small(BC, V_USED)
        for c in range(4):
            mm(v_ps[:], hT3[:, c, :], w2_sb[:, c, :],
               start=(c == 0), stop=False)
        mm(v_ps[:], ones_1x2[:], b2_sb[:], start=False, stop=True)
        v_sb = sb(BC, V_USED, "v_sb")
        dve.tensor_copy(v_sb[:], v_ps[:])

        # ---- gates (sigmoid table cluster, batched [BC, w]) ----
        er_sg = sb(BC, WD, "er_sg")
        act.activation(er_sg[:], v_sb[:, O_ER:O_ER + WD], AF.Sigmoid)
        fg_sg = sb(BC, R, "fg_sg")
        act.activation(fg_sg[:], v_sb[:, O_FG:O_FG + R], AF.Sigmoid)
        awg = sb(BC, 2, "awg")      # [alloc_gate, write_gate]
        act.activation(awg[:], v_sb[:, O_AG:O_AG + 2], AF.Sigmoid)

        # ---- exp/ln table from here on ----
        rm_e = sb(BC, 3 * R, "rm_e")
        act.activation(rm_e[:], v_sb[:, O_RM:O_RM + 3 * R], AF.Exp)
        rm_sum = sb(BC, R, "rm_sum")
        dve.tensor_reduce(rm_sum[:], rm_e[:].rearrange("o (r t) -> o r t", t=3),
                          axis=mybir.AxisListType.X, op=OP.add)
        rm_rec = sb(BC, R, "rm_rec")
        dve.reciprocal(rm_rec[:], rm_sum[:])
        modes = sb(BC, 3 * R, "modes")
        dve.tensor_tensor(modes[:].rearrange("o (r t) -> o r t", t=3),
                          rm_e[:].rearrange("o (r t) -> o r t", t=3),
                          rm_rec[:].rearrange("o (r t) -> o r t", t=1)
                          .broadcast_to([BC, R, 3]),
                          op=OP.mult)

        # usage scalar u and gate combos, batched
        fgN = sb(BC, R, "fgN")
        act.activation(fgN[:], fg_sg[:], AF.Copy, scale=-1.0 / N, bias=1.0)
        fg2 = sb(BC, 2, "fg2")
        dve.tensor_tensor(fg2[:], fgN[:, 0:2], fgN[:, 2:4], op=OP.mult)
        prod = sb(BC, 1, "prod")
        dve.tensor_tensor(prod[:], fg2[:, 0:1], fg2[:, 1:2], op=OP.mult)
        u_sb = sb(BC, 1, "u_sb")
        act.activation(u_sb[:], prod[:], AF.Copy, scale=1e-4)
        ln_u = sb(BC, 1, "ln_u")
        act.activation(ln_u[:], u_sb[:], AF.Ln)
        omu = sb(BC, 1, "omu")
        act.activation(omu[:], u_sb[:], AF.Copy, scale=-1.0, bias=1.0)
        omag = sb(BC, 1, "omag")
        act.activation(omag[:], awg[:, 0:1], AF.Copy, scale=-1.0, bias=1.0)
        c1 = sb(BC, 1, "c1")
        dve.tensor_tensor(c1[:], awg[:, 1:2], awg[:, 0:1], op=OP.mult)
        c2 = sb(BC, 1, "c2")
        dve.tensor_tensor(c2[:], awg[:, 1:2], omag[:], op=OP.mult)

        # normalized write keys, batched: kf = exp(-0.5 ln(|k|^2))
        wk2 = sb(BC, 1, "wk2")
        s64 = scr.tile([BC, WD], F32, tag="s64", name="s64")
        act.activation(s64[:], v_sb[:, O_WK:O_WK + WD], AF.Square,
                       accum_out=wk2[:])
        wf = sb(BC, 1, "wf")
        act.activation(wf[:], wk2[:], AF.Ln)
        act.activation(wf[:], wf[:], AF.Exp, scale=-0.5)
        kn = sb(BC, WD, "kn")
        act.activation(kn[:], v_sb[:, O_WK:O_WK + WD], AF.Copy, scale=wf[:])

        # normalized read keys, batched
        rk2 = sb(BC, R, "rk2")
        for r in range(R):
            s64r = scr.tile([BC, WD], F32, tag="s64r", name="s64r")
            act.activation(s64r[:], v_sb[:, O_RK + WD * r:O_RK + WD * (r + 1)],
                           AF.Square, accum_out=rk2[:, r:r + 1])
        rf = sb(BC, R, "rf")
        act.activation(rf[:], rk2[:], AF.Ln)
        act.activation(rf[:], rf[:], AF.Exp, scale=-0.5)
        rkn = sb(BC, R * WD, "rkn")
        dve.tensor_tensor(rkn[:].rearrange("o (r w) -> o r w", w=WD),
                          v_sb[:, O_RK:O_RK + R * WD]
                          .rearrange("o (r w) -> o r w", w=WD),
                          rf[:].rearrange("o (r w) -> o r w", w=1)
                          .broadcast_to([BC, R, WD]),
                          op=OP.mult)

        # batched packs, unbatched later via selector matmuls
        sc4 = sb(BC, 4, "sc4")          # [ln_u, 1-u, c1, c2]
        dve.tensor_copy(sc4[:, 0:1], ln_u[:])
        dve.tensor_copy(sc4[:, 1:2], omu[:])
        dve.tensor_copy(sc4[:, 2:3], c1[:])
        dve.tensor_copy(sc4[:, 3:4], c2[:])
        ev2 = sb(BC, 2 * WD, "ev2")     # [erase | write_vector]
        dve.tensor_copy(ev2[:, 0:WD], er_sg[:])
        dve.tensor_copy(ev2[:, WD:2 * WD], v_sb[:, O_WV:O_WV + WD])

        # =========== per-batch addressing (w chain first) ===========
        st = [dict() for _ in range(BC)]
        for b in range(BC):
            s = st[b]
            M3 = M_sb[b][:].rearrange("q (i w) -> q i w", w=WD)

            # M row norms: rn = exp(-0.5 ln(|M_n|^2))
            msq = sb(128, NCH, f"msq{b}")
            sq1 = sqp.tile([128, NCH * WD], F32, tag="sq1", name="sq1")
            dve.tensor_tensor(sq1[:], M_sb[b][:], M_sb[b][:], op=OP.mult)
            dve.tensor_reduce(msq[:], sq1[:].rearrange(
                "q (i w) -> q i w", w=WD), axis=mybir.AxisListType.X,
                op=OP.add)
            rn_w = sb(128, NCH, f"rn_w{b}")
            act.activation(rn_w[:], msq[:], AF.Ln)
            act.activation(rn_w[:], rn_w[:], AF.Exp, scale=-0.5)

            kn_bc = sb(128, WD, f"kn_bc{b}")
            ptk = ps_small(128, WD)
            mm(ptk[:], sel2[:, 128 * b:128 * (b + 1)], kn[:])
            dve.tensor_copy(kn_bc[:], ptk[:])

            wsc_r = sb(128, NCH, f"wsc_r{b}")
            g64 = scr.tile([128, NCH * WD], BF16, tag="g64", name="g64")
            for i in range(NCH):
                dve.scalar_tensor_tensor(
                    out=g64[:, WD * i:WD * (i + 1)], in0=M3[:, i, :],
                    scalar=1.0, in1=kn_bc[:], op0=OP.mult, op1=OP.mult,
                    accum_out=wsc_r[:, i:i + 1])
            wsc = sb(128, NCH, f"wsc{b}")
            dve.tensor_tensor(wsc[:], wsc_r[:], rn_w[:], op=OP.mult)

            wse = sb(128, NCH, f"wse{b}")
            wse_s = sb(128, 1, f"wse_s{b}")
            act.activation(wse[:], wsc[:], AF.Exp, accum_out=wse_s[:])
            ptt = ps_small(1, 1)
            mm(ptt[:], wse_s[:], ones_col[:])
            totr = sb(1, 1, f"totr{b}")
            dve.reciprocal(totr[:], ptt[:])

            # [ln_u, 1-u, c1, c2] broadcast to 128 parts; totr separately
            pb4 = ps_small(128, 4)
            mm(pb4[:], sel2[:, 128 * b:128 * (b + 1)], sc4[:])
            scb = sb(128, 4, f"scb{b}")
            dve.tensor_copy(scb[:], pb4[:])
            ptb2 = ps_small(128, 1)
            mm(ptb2[:], ones_row[:], totr[:])
            totb = sb(128, 1, f"totb{b}")
            dve.tensor_copy(totb[:], ptb2[:])

            alle = sb(128, NCH, f"alle{b}")
            act.activation(alle[:], iota[:], AF.Exp, scale=scb[:, 0:1])
            alloc = sb(128, NCH, f"alloc{b}")
            act.activation(alloc[:], alle[:], AF.Copy, scale=scb[:, 1:2])

            cww = sb(128, NCH, f"cww{b}")
            dve.tensor_scalar_mul(cww[:], wse[:], totb[:])
            t2 = sb(128, NCH, f"t2w{b}")
            dve.tensor_scalar_mul(t2[:], cww[:], scb[:, 3:4])
            w_sb = sb(128, NCH, f"w_sb{b}")
            dve.scalar_tensor_tensor(out=w_sb[:], in0=alloc[:],
                                     scalar=scb[:, 2:3], in1=t2[:],
                                     op0=OP.mult, op1=OP.add)
            s['w_sb'] = w_sb

            # stream lhsT: [ones|w] in this batch's column pair, zeros in
            # the other batch's, so both batches share one [4,N] psum group
            oww = bpool.tile([128, 4 * NCH], BF16, tag=f"oww{b}",
                             name="oww")
            oww3 = oww[:].rearrange("q (i t) -> q i t", t=4)
            dve.memset(oww[:], 0.0)
            dve.memset(oww3[:, :, 2 * b], 1.0)
            dve.tensor_copy(oww3[:, :, 2 * b + 1], w_sb[:])
            s['oww3'] = oww3

            wrow = bone.tile([1, N], BF16, tag=f"wrow{b}", name="wrow")
            w_bc = bone.tile([128, N], BF16, tag=f"w_bc{b}", name="w_bc")
            for g in range(4):
                pr = ps_small(1, 512)
                for j in range(4):
                    c = 4 * g + j
                    mm(pr[0:1, 128 * j:128 * (j + 1)], w_sb[:, c:c + 1],
                       i128[:])
                dve.tensor_copy(wrow[0:1, 512 * g:512 * (g + 1)], pr[:])
                pb = ps_small(128, 512)
                mm(pb[:], ones_row_bf[:], wrow[0:1, 512 * g:512 * (g + 1)])
                act.copy(w_bc[:, 512 * g:512 * (g + 1)], pb[:])
            s['w_bc'] = w_bc
            s['wrow'] = wrow

            # W = sum(w), P = sum(p) broadcast [128, 2]
            wsum = sb(1, 1, f"wsum{b}")
            pws = ps_small(1, NCH)
            mm(pws[:], ones_col[:], w_sb[:])
            ws16 = sb(1, NCH, f"ws16{b}")
            dve.tensor_copy(ws16[:], pws[:])
            dve.tensor_reduce(wsum[:], ws16[:], axis=mybir.AxisListType.X,
                              op=OP.add)
            psum_s = sb(1, 1, f"psum_s{b}")
            pps = ps_small(1, NCH)
            mm(pps[:], ones_col[:], pT[b][:])
            ps16 = sb(1, NCH, f"ps16{b}")
            dve.tensor_copy(ps16[:], pps[:])
            dve.tensor_reduce(psum_s[:], ps16[:], axis=mybir.AxisListType.X,
                              op=OP.add)
            pw2 = sb(1, 2, f"pw2{b}")
            dve.tensor_copy(pw2[0:1, 0:1], psum_s[:])
            dve.tensor_copy(pw2[0:1, 1:2], wsum[:])
            pbx = ps_small(128, 2)
            mm(pbx[:], ones_row[:], pw2[:])
            pwb = sb(128, 2, f"pwb{b}")
            dve.tensor_copy(pwb[:], pbx[:])

            # endgame precomputes that need only w and p
            def bcol(col):
                return col.rearrange("q (a o) -> q a o", a=1).broadcast_to(
                    [128, 1, NCH])[:, 0, :]
            omw = sb(128, NCH, f"omw{b}")
            act.activation(omw[:], w_sb[:], AF.Copy, scale=-1.0, bias=1.0)
            r_t1 = sb(128, NCH, f"r_t1{b}")
            gp.tensor_tensor(r_t1[:], bcol(pwb[:, 0:1]), pT[b][:],
                             op=OP.subtract)
            r_t2 = sb(128, NCH, f"r_t2{b}")
            gp.tensor_tensor(r_t2[:], w_sb[:], r_t1[:], op=OP.mult)
            c_t1 = sb(128, NCH, f"c_t1{b}")
            gp.tensor_tensor(c_t1[:], bcol(pwb[:, 1:2]), w_sb[:],
                             op=OP.subtract)
            c_t2 = sb(128, NCH, f"c_t2{b}")
            gp.tensor_tensor(c_t2[:], pT[b][:], c_t1[:], op=OP.mult)
            s['omw'], s['r_t2'], s['c_t2'] = omw, r_t2, c_t2

            # stream accumulator targets
            s['rs0'] = sb(128, NCH, f"rs0{b}")
            s['lw'] = sb(128, NCH, f"lw{b}")

        # =========== memory update + read scores (overlap the stream) =====
        for b in range(BC):
            s = st[b]
            M3 = M_sb[b][:].rearrange("q (i w) -> q i w", w=WD)
            evp = ps_small(1, 2 * WD)
            mm(evp[:], i128[0:BC, b:b + 1], ev2[:])
            ev = bpool.tile([1, 2 * WD], BF16, tag=f"ev{b}", name="ev")
            dve.tensor_copy(ev[:], evp[:])

            FG = bone.tile([128, NCH * 2 * WD], F32, tag=f"FG{b}", name="FG")
            FG3 = FG[:].rearrange("q (i w) -> q i w", w=2 * WD)
            for g in range(4):
                ptf = ps_small(128, 512)
                pt4 = ptf[:].rearrange("q (j w) -> q j w", w=2 * WD)
                for j in range(4):
                    i = 4 * g + j
                    mm(pt4[:, j, :], s['wrow'][0:1, 128 * i:128 * (i + 1)],
                       ev[:])
                # F = 1 - w x e ; G = w x v
                dve.scalar_tensor_tensor(
                    out=FG3[:, 4 * g:4 * (g + 1), 0:WD],
                    in0=pt4[:, :, 0:WD], scalar=-1.0,
                    in1=ones256[:].rearrange("q (j w) -> q j w", w=WD),
                    op0=OP.mult, op1=OP.add)
                dve.tensor_copy(FG3[:, 4 * g:4 * (g + 1), WD:2 * WD],
                                pt4[:, :, WD:2 * WD])

            Mn_sb = bone.tile([128, NCH * WD], F32, tag=f"Mn{b}", name="Mn")
            Mn3 = Mn_sb[:].rearrange("q (i w) -> q i w", w=WD)
            for g in range(8):
                i0, i1 = 2 * g, 2 * g + 2
                g1 = scr.tile([128, 2 * WD], F32, tag="gmn", name="gmn")
                gp.tensor_tensor(g1[:].rearrange("q (i w) -> q i w", w=WD),
                                 M3[:, i0:i1, :], FG3[:, i0:i1, 0:WD],
                                 op=OP.mult)
                gp.tensor_tensor(Mn3[:, i0:i1, :],
                                 g1[:].rearrange("q (i w) -> q i w", w=WD),
                                 FG3[:, i0:i1, WD:2 * WD], op=OP.add)
            s['Mn3'] = Mn3
            s['Mn_sb'] = Mn_sb

            mq2 = sb(128, NCH, f"mq2{b}")
            sq2 = sqp.tile([128, NCH * WD], F32, tag="sq1", name="sq1")
            dve.tensor_tensor(sq2[:], Mn_sb[:], Mn_sb[:], op=OP.mult)
            dve.tensor_reduce(mq2[:], sq2[:].rearrange(
                "q (i w) -> q i w", w=WD), axis=mybir.AxisListType.X,
                op=OP.add)
            rn2 = sb(128, NCH, f"rn2{b}")
            act.activation(rn2[:], mq2[:], AF.Ln)
            act.activation(rn2[:], rn2[:], AF.Exp, scale=-0.5)

            MnT = bone.tile([64, NCH * 128], BF16, tag=f"MnT{b}", name="MnT")
            MnT3 = MnT[:].rearrange("q (i c) -> q i c", c=128)
            for g in range(4):
                ptm = ps_small(64, 512)
                for j in range(4):
                    pe.transpose(ptm[:, 128 * j:128 * (j + 1)],
                                 Mn3[:, 4 * g + j, :], i128[:])
                act.copy(MnT[0:64, 512 * g:512 * (g + 1)], ptm[:])

            rknp = ps_small(1, R * WD)
            mm(rknp[:], i128[0:BC, b:b + 1], rkn[:])
            rkb = sb(1, R * WD, f"rkb{b}")
            dve.tensor_copy(rkb[:], rknp[:])
            rknT = bpool.tile([64, R], BF16, tag=f"rknT{b}", name="rknT")
            ptk2 = ps_small(64, R)
            for r in range(R):
                mm(ptk2[:, r:r + 1],
                   rkb[0:1, WD * r:WD * (r + 1)],
                   ones_1x2[0:1, 0:1])
            dve.tensor_copy(rknT[:], ptk2[:])

            rsc = sb(128, R * NCH, f"rsc{b}")
            rsc3 = rsc[:].rearrange("q (r i) -> q r i", i=NCH)
            for i in range(NCH):
                ptr = ps_small(128, R)
                mm(ptr[:], MnT3[:, i, :], rknT[:])
                dve.tensor_scalar_mul(rsc3[:, :, i], ptr[:], rn2[:, i:i + 1])
            rex = sb(128, R * NCH, f"rex{b}")
            rex3 = rex[:].rearrange("q (r i) -> q r i", i=NCH)
            res_s = sb(128, R, f"res_s{b}")
            for r in range(R):
                act.activation(rex3[:, r, :], rsc3[:, r, :], AF.Exp,
                               accum_out=res_s[:, r:r + 1])
            ptot = ps_small(R, 1)
            mm(ptot[:], res_s[:], ones_col[:])
            rec4 = sb(R, 1, f"rec4{b}")
            dve.reciprocal(rec4[:], ptot[:])
            prr = ps_small(1, R)
            mm(prr[:], rec4[:], i128[0:R, 0:R])
            rec_row = sb(1, R, f"rec_row{b}")
            dve.tensor_copy(rec_row[:], prr[:])
            s['rex3'] = rex3
            s['rec_row'] = rec_row

        # =========== the L stream: both batches interleaved ===========
        cscw_ps = pbig.tile([4, N], F32, tag="cscw", name="cscw")
        for i in range(NCH):
            for b in range(BC):
                s = st[b]
                lblk = lpool.tile([128, N], F32, tag="lblk", name="lblk")
                nc.sync.dma_start(lblk[:], l_ap[b, 128 * i:128 * (i + 1), :])
                lb = lbf.tile([128, N], BF16, tag="lbf", name="lbf")
                act.activation(lb[:], lblk[:], AF.Copy,
                               accum_out=s['rs0'][:, i:i + 1])
                for c in range(4):
                    mm(cscw_ps[:, 512 * c:512 * (c + 1)],
                       s['oww3'][:, i, :], lb[:, 512 * c:512 * (c + 1)],
                       start=(i == 0 and b == 0),
                       stop=(i == NCH - 1 and b == BC - 1))
                sT = scr.tile([128, N], BF16, tag="sttr", name="sttr")
                dve.scalar_tensor_tensor(
                    out=sT[:], in0=lblk[:], scalar=1.0, in1=s['w_bc'][:],
                    op0=OP.mult, op1=OP.mult,
                    accum_out=s['lw'][:, i:i + 1])

        # =========== endgame: shared colsum readout, then per batch =====
        cscw_sb = bone.tile([4, N], F32, tag="cscw_sb", name="cscw_sb")
        act.copy(cscw_sb[:], cscw_ps[:])
        csT = bone.tile([128, 4 * NCH], F32, tag="csT", name="csT")
        csT3 = csT[:].rearrange("q (i t) -> q i t", t=4)
        ptc = ps_small(128, 4 * NCH)
        for c in range(NCH):
            mm(ptc[:, 4 * c:4 * c + 4],
               cscw_sb[0:4, 128 * c:128 * (c + 1)], i128[0:4, 0:4])
        dve.tensor_copy(csT[:], ptc[:])
        for b in range(BC):
            s = st[b]
            cs0T = csT3[:, :, 2 * b]
            cwT = csT3[:, :, 2 * b + 1]

            # rowsum_Lnew/N -> exp ; colsum_Lnew/N -> exp
            rr1 = sb(128, NCH, f"rr1{b}")
            gp.tensor_tensor(rr1[:], s['omw'][:], s['rs0'][:], op=OP.mult)
            gp.tensor_tensor(rr1[:], rr1[:], s['lw'][:], op=OP.subtract)
            gp.tensor_tensor(rr1[:], rr1[:], s['r_t2'][:], op=OP.add)
            ebw = sb(128, NCH, f"ebw{b}")
            ebw_s = sb(128, 1, f"ebw_s{b}")
            act.activation(ebw[:], rr1[:], AF.Exp, scale=1.0 / N,
                           accum_out=ebw_s[:])
            cc1 = sb(128, NCH, f"cc1{b}")
            gp.tensor_tensor(cc1[:], s['omw'][:], cs0T, op=OP.mult)
            gp.tensor_tensor(cc1[:], cc1[:], cwT, op=OP.subtract)
            gp.tensor_tensor(cc1[:], cc1[:], s['c_t2'][:], op=OP.add)
            efw = sb(128, NCH, f"efw{b}")
            efw_s = sb(128, 1, f"efw_s{b}")
            act.activation(efw[:], cc1[:], AF.Exp, scale=1.0 / N,
                           accum_out=efw_s[:])

            ptb = ps_small(1, 2)
            mm(ptb[0:1, 0:1], ebw_s[:], ones_col[:])
            mm(ptb[0:1, 1:2], efw_s[:], ones_col[:])
            rec_bf = sb(1, 2, f"rec_bf{b}")
            dve.reciprocal(rec_bf[:], ptb[:])

            # per-head coefficients folded with softmax normalizers
            mptr = ps_small(1, 3 * R)
            mm(mptr[:], i128[0:BC, b:b + 1], modes[:])
            mo_b = sb(1, 3 * R, f"mo_b{b}")
            dve.tensor_copy(mo_b[:], mptr[:])
            bvec = sb(1, 3 * R, f"bvec{b}")
            m3v = mo_b[:].rearrange("o (r t) -> o r t", t=3)
            dve.tensor_tensor(bvec[0:1, 0:R], m3v[:, :, 0],
                              rec_bf[0:1, 0:1].broadcast_to([1, R]),
                              op=OP.mult)
            dve.tensor_tensor(bvec[0:1, R:2 * R], m3v[:, :, 1],
                              s['rec_row'][:], op=OP.mult)
            dve.tensor_tensor(bvec[0:1, 2 * R:3 * R], m3v[:, :, 2],
                              rec_bf[0:1, 1:2].broadcast_to([1, R]),
                              op=OP.mult)
            pbv = ps_small(128, 3 * R)
            mm(pbv[:], ones_row[:], bvec[:])
            Bco = sb(128, 3 * R, f"Bco{b}")
            dve.tensor_copy(Bco[:], pbv[:])
            B3 = Bco[:].rearrange("q (t r) -> q t r", r=R)

            # read weights: rw[q, r, i] = B0_r*ebw + B1_r*rex + B2_r*efw
            rw_sb = sb(128, R * NCH, f"rw_sb{b}")
            rw3 = rw_sb[:].rearrange("q (r i) -> q r i", i=NCH)
            ebw_b = ebw[:].rearrange("q (a i) -> q a i", a=1).broadcast_to(
                [128, R, NCH])
            efw_b = efw[:].rearrange("q (a i) -> q a i", a=1).broadcast_to(
                [128, R, NCH])
            z1 = sb(128, R * NCH, f"z1{b}")
            z13 = z1[:].rearrange("q (r i) -> q r i", i=NCH)
            gp.tensor_tensor(
                rw3[:], ebw_b,
                B3[:, 0, :].rearrange("q (r a) -> q r a", a=1)
                .broadcast_to([128, R, NCH]), op=OP.mult)
            gp.tensor_tensor(
                z13[:], s['rex3'][:],
                B3[:, 1, :].rearrange("q (r a) -> q r a", a=1)
                .broadcast_to([128, R, NCH]), op=OP.mult)
            gp.tensor_tensor(rw3[:], rw3[:], z13[:], op=OP.add)
            gp.tensor_tensor(
                z13[:], efw_b,
                B3[:, 2, :].rearrange("q (r a) -> q r a", a=1)
                .broadcast_to([128, R, NCH]), op=OP.mult)
            gp.tensor_tensor(rw3[:], rw3[:], z13[:], op=OP.add)

            prv = pacc.tile([R, WD], F32, tag="prv", name="prv")
            rw_by_i = rw_sb[:].rearrange("q (r i) -> q i r", i=NCH)
            for i in range(NCH):
                mm(prv[:], rw_by_i[:, i, :], s['Mn3'][:, i, :],
                   start=(i == 0), stop=(i == NCH - 1))
            out_sb = sb(R, WD, f"out_sb{b}")
            dve.tensor_copy(out_sb[:], prv[:])
            nc.sync.dma_start(out_ap[b], out_sb[:])

    nc.compile()
    return nc


_NC_CACHE = []


def kernel(x, memory, L, p, W1, b1, W2, b2):
    x = np.ascontiguousarray(x, np.float32)
    memory = np.ascontiguousarray(memory, np.float32)
    L = np.ascontiguousarray(L, np.float32)
    p = np.ascontiguousarray(p, np.float32)
    W1 = np.ascontiguousarray(W1, np.float32)
    b1 = np.ascontiguousarray(b1, np.float32).reshape(1, H_D)
    W2 = np.ascontiguousarray(W2, np.float32)
    b2 = np.ascontiguousarray(b2, np.float32).reshape(1, IFACE)

    iota = (np.arange(N, dtype=np.float32).reshape(NCH, 128).T + 1.0).copy()
    i128 = np.eye(128, dtype=np.float32)
    sel2 = np.zeros((BC, BC * 128), dtype=np.float32)
    for b in range(BC):
        sel2[b, 128 * b:128 * (b + 1)] = 1.0

    if not _NC_CACHE:
        _NC_CACHE.append(build_nc())
    nc = _NC_CACHE[0]

    in_maps = []
    for c in range(NCORES):
        s = slice(BC * c, BC * (c + 1))
        in_maps.append({
            'x': x[s], 'memory': memory[s], 'L': L[s], 'p': p[s],
            'W1': W1, 'b1': b1, 'W2': W2, 'b2': b2,
            'iota_p1': iota, 'i128': i128, 'sel2': sel2,
        })

    res = run_bass_kernel_spmd(nc, in_maps, list(range(NCORES)))
    outs = [res.results[c]['out'].reshape(BC, 1, R * WD)
            for c in range(NCORES)]
    return np.concatenate(outs, axis=0)


# revision 35
# speedup vs baseline: 1.2075x; 1.0034x over previous
"""DNC forward (single step) on 8 NeuronCores — Bass/Tile kernel.

Data parallel: 16 batches -> 2 per core. Algebraic facts exploited (valid
for the prev_state==None path of the reference):

* prev_rw is uniform (1/N)  => fwd/bwd temporal read weights only need the
  row-sums and column-sums of L_new, never L_new itself.  With
  rowsum0 = L@1, Lw = L@w, colsum0 = 1@L, cw = w@L (w = write weights):
      rowsum_Lnew = (1-w)*rowsum0 - Lw + w*(sum(p) - p)
      colsum_Lnew = (1-w)*colsum0 - cw + p*(sum(w) - w)
  so L is streamed exactly once from HBM (the memory-bound roofline).
* var_phi / usage are constant across slots => argsort is the identity and
  allocation[n] = (1-u) * u^(n+1) with u = 1e-4 * prod_r(1 - free_gate_r/N).
* read/write strengths cancel inside the cosine normalization (mod the 1e-8
  eps guard), so the softplus chains are dropped.
* 1/(sqrt(x)+eps) -> exp(-0.5*ln(x)); all Ln ops are clustered so the ACT
  function-table loads stay at ~5 for the whole kernel.

Schedule: DMA order is x/W1/consts/M/W2/p then the two batches' L streams
interleaved block-by-block.  Per 1 MB row-block of L, the stream consumers
run on three engines (ACT copy+rowsum / PE colsum psum / DVE-or-Pool
weighted reduce); every fourth block's weighted reduce runs on the Pool
engine to keep DVE below the DMA pace.  The memory update + content read
scores are emitted as background tasks interleaved into the stream loop so
the in-order engines absorb them in their per-block slack.
Both batches' colsum chains share one [4,N] psum accumulation group via
zero-padded 4-column lhsT ([ones|w|0|0] vs [0|0|ones|w]).
Slot layout: n = 128*i + q (partition q, chunk i).
"""
import numpy as np
from contextlib import ExitStack

import concourse.bass as bass
import concourse.bacc as bacc
import concourse.tile as tile
from concourse import mybir
from concourse.bass_utils import run_bass_kernel_spmd

F32 = mybir.dt.float32
BF16 = mybir.dt.bfloat16
AF = mybir.ActivationFunctionType
OP = mybir.AluOpType

NCORES = 8
BC = 2                  # batches per core
N = 2048                # memory slots
NCH = N // 128          # 16 slot chunks
WD = 64                 # word size
R = 4                   # read heads
IN_D, H_D, IFACE = 256, 512, 727
V_USED = 471            # interface cols actually used (output_vector is dead)

# interface vector slice offsets
O_RK, O_RS, O_WK, O_WS = 0, 256, 260, 324
O_ER, O_WV, O_FG, O_AG, O_WG, O_RM = 325, 389, 453, 457, 458, 459


def build_nc():
    nc = bacc.Bacc("TRN2", target_bir_lowering=False, debug=False)

    x_ap = nc.dram_tensor("x", [BC, IN_D], BF16, kind="ExternalInput").ap()
    mem_ap = nc.dram_tensor("memory", [BC, N, WD], F32,
                            kind="ExternalInput").ap()
    l_ap = nc.dram_tensor("L", [BC, N, N], F32, kind="ExternalInput").ap()
    p_ap = nc.dram_tensor("p", [BC, 1, N], F32, kind="ExternalInput").ap()
    w1_ap = nc.dram_tensor("W1", [IN_D, H_D], BF16, kind="ExternalInput").ap()
    b1_ap = nc.dram_tensor("b1", [1, H_D], BF16, kind="ExternalInput").ap()
    w2_ap = nc.dram_tensor("W2", [H_D, IFACE], BF16,
                           kind="ExternalInput").ap()
    b2_ap = nc.dram_tensor("b2", [1, IFACE], BF16, kind="ExternalInput").ap()
    iota_ap = nc.dram_tensor("iota_p1", [128, NCH], F32,
                             kind="ExternalInput").ap()
    i128_ap = nc.dram_tensor("i128", [128, 128], F32, kind="ExternalInput").ap()
    sel2_ap = nc.dram_tensor("sel2", [BC, BC * 128], F32,
                             kind="ExternalInput").ap()
    out_ap = nc.dram_tensor("out", [BC, R, WD], F32,
                            kind="ExternalOutput").ap()

    with tile.TileContext(nc) as tc, ExitStack() as ctx:
        act = nc.scalar
        dve = nc.vector
        gp = nc.gpsimd
        pe = nc.tensor

        persist = ctx.enter_context(tc.tile_pool(name="persist", bufs=1))
        bpool = ctx.enter_context(tc.tile_pool(name="bpool", bufs=2))
        lpool = ctx.enter_context(tc.tile_pool(name="lpool", bufs=9))
        lbf = ctx.enter_context(tc.tile_pool(name="lbf", bufs=4))
        scr = ctx.enter_context(tc.tile_pool(name="scr", bufs=1))
        bone = ctx.enter_context(tc.tile_pool(name="bone", bufs=1))
        sqp = ctx.enter_context(tc.tile_pool(name="sqp", bufs=2))
        pss = ctx.enter_context(tc.tile_pool(name="pss", bufs=2, space="PSUM"))
        pacc = ctx.enter_context(tc.tile_pool(name="pacc", bufs=2,
                                              space="PSUM"))
        pbig = ctx.enter_context(tc.tile_pool(name="pbig", bufs=1,
                                              space="PSUM"))

        def mm(out, lhsT, rhs, start=True, stop=True):
            pe.matmul(out, lhsT, rhs, start=start, stop=stop)

        def ps_small(p_, f):
            return pss.tile([p_, f], F32, tag="pss", name="pss")

        def sb(p_, f, tag):
            return bpool.tile([p_, f], F32, tag=tag, name=tag)

        # ---- constants + weights (DMA order = transfer order) ----
        ones_row = persist.tile([1, 128], F32, tag="ones_row")
        dve.memset(ones_row[:], 1.0)
        ones_col = persist.tile([128, 1], F32, tag="ones_col")
        dve.memset(ones_col[:], 1.0)
        ones_1x2 = persist.tile([1, 2], BF16, tag="ones_1x2")
        dve.memset(ones_1x2[:], 1.0)
        one_f32 = persist.tile([1, 2], F32, tag="one_f32")
        dve.memset(one_f32[:], 1.0)
        ones_row_bf = persist.tile([1, 128], BF16, tag="ones_row_bf")
        dve.memset(ones_row_bf[:], 1.0)
        ones256 = persist.tile([128, 256], F32, tag="ones256")
        dve.memset(ones256[:], 1.0)
        i2bf = persist.tile([BC, BC], BF16, tag="i2bf")

        xb = persist.tile([BC, IN_D], BF16, tag="xb")
        nc.sync.dma_start(xb[:], x_ap[:, :])
        w1_sb = persist.tile([128, 2, H_D], BF16, tag="w1_sb")
        for c in range(2):
            nc.sync.dma_start(w1_sb[:, c, :], w1_ap[128 * c:128 * (c + 1), :])
        b1_sb = persist.tile([1, H_D], BF16, tag="b1_sb")
        nc.sync.dma_start(b1_sb[:], b1_ap)
        b2_sb = persist.tile([1, V_USED], BF16, tag="b2_sb")
        nc.sync.dma_start(b2_sb[:], b2_ap[0:1, 0:V_USED])
        i128 = persist.tile([128, 128], F32, tag="i128")
        nc.sync.dma_start(i128[:], i128_ap)
        iota = persist.tile([128, NCH], F32, tag="iota")
        nc.sync.dma_start(iota[:], iota_ap)
        sel2 = persist.tile([BC, BC * 128], F32, tag="sel2")
        nc.sync.dma_start(sel2[:], sel2_ap)
        dve.tensor_copy(i2bf[:], i128[0:BC, 0:BC])

        w2_sb = persist.tile([128, 4, V_USED], BF16, tag="w2_sb")
        for c in range(4):
            nc.sync.dma_start(w2_sb[:, c, :],
                              w2_ap[128 * c:128 * (c + 1), 0:V_USED])
        M_sb = []
        for b in range(BC):
            Mb = bone.tile([128, NCH * WD], F32, tag=f"M_sb{b}", name="M_sb")
            nc.sync.dma_start(Mb[:].rearrange("q (i w) -> q i w", w=WD),
                              mem_ap[b].rearrange("(i q) w -> q i w", q=128))
            M_sb.append(Mb)
        pT = []
        for b in range(BC):
            pb = bpool.tile([128, NCH], F32, tag="pT", name="pT")
            nc.sync.dma_start(
                pb[:].rearrange("q (c o) -> q c o", o=1),
                p_ap[b, 0:1, :].rearrange("o (c q) -> q c o", q=128))
            pT.append(pb)

        # =========== batched controller (both batches at once) ===========
        xT = bpool.tile([128, 2 * BC], BF16, tag="xT", name="xT")
        xT3 = xT[:].rearrange("q (c b) -> q c b", b=BC)
        ptx = pss.tile([128, 2 * BC], BF16, tag="pss", name="pss")
        for c in range(2):
            pe.transpose(ptx[:, BC * c:BC * (c + 1)],
                         xb[0:BC, 128 * c:128 * (c + 1)], i2bf[:])
        dve.tensor_copy(xT[:], ptx[:])

        h_ps = ps_small(BC, H_D)
        for c in range(2):
            mm(h_ps[:], xT3[:, c, :], w1_sb[:, c, :],
               start=(c == 0), stop=False)
        mm(h_ps[:], ones_1x2[:], b1_sb[:], start=False, stop=True)
        h_sb = bpool.tile([BC, H_D], BF16, tag="h_sb", name="h_sb")
        act.activation(h_sb[:], h_ps[:], AF.Tanh)

        hT = bpool.tile([128, 4 * BC], BF16, tag="hT", name="hT")
        hT3 = hT[:].rearrange("q (c b) -> q c b", b=BC)
        pth = pss.tile([128, 4 * BC], BF16, tag="pss", name="pss")
        for c in range(4):
            pe.transpose(pth[:, BC * c:BC * (c + 1)],
                         h_sb[0:BC, 128 * c:128 * (c + 1)], i2bf[:])
        dve.tensor_copy(hT[:], pth[:])

        v_ps = ps_small(BC, V_USED)
        for c in range(4):
            mm(v_ps[:], hT3[:, c, :], w2_sb[:, c, :],
               start=(c == 0), stop=False)
        mm(v_ps[:], ones_1x2[:], b2_sb[:], start=False, stop=True)
        v_sb = sb(BC, V_USED, "v_sb")
        dve.tensor_copy(v_sb[:], v_ps[:])

        # ---- sigmoid-table cluster (batched [BC, w]) ----
        er_sg = sb(BC, WD, "er_sg")
        act.activation(er_sg[:], v_sb[:, O_ER:O_ER + WD], AF.Sigmoid)
        fg_sg = sb(BC, R, "fg_sg")
        act.activation(fg_sg[:], v_sb[:, O_FG:O_FG + R], AF.Sigmoid)
        awg = sb(BC, 2, "awg")      # [alloc_gate, write_gate]
        act.activation(awg[:], v_sb[:, O_AG:O_AG + 2], AF.Sigmoid)

        # ---- pre-Ln work (Square/Copy are in every table set) ----
        wk2 = sb(BC, 1, "wk2")
        s64 = scr.tile([BC, WD], F32, tag="s64", name="s64")
        act.activation(s64[:], v_sb[:, O_WK:O_WK + WD], AF.Square,
                       accum_out=wk2[:])
        rk2 = sb(BC, R, "rk2")
        for r in range(R):
            s64r = scr.tile([BC, WD], F32, tag="s64r", name="s64r")
            act.activation(s64r[:], v_sb[:, O_RK + WD * r:O_RK + WD * (r + 1)],
                           AF.Square, accum_out=rk2[:, r:r + 1])

        fgN = sb(BC, R, "fgN")
        act.activation(fgN[:], fg_sg[:], AF.Copy, scale=-1.0 / N, bias=1.0)
        fg2 = sb(BC, 2, "fg2")
        dve.tensor_tensor(fg2[:], fgN[:, 0:2], fgN[:, 2:4], op=OP.mult)
        prod = sb(BC, 1, "prod")
        dve.tensor_tensor(prod[:], fg2[:, 0:1], fg2[:, 1:2], op=OP.mult)
        u_sb = sb(BC, 1, "u_sb")
        act.activation(u_sb[:], prod[:], AF.Copy, scale=1e-4)

        # M squared row norms via Pool (keeps DVE free)
        msq, rn_w = [], []
        for b in range(BC):
            mq = sb(128, NCH, f"msq{b}")
            gsq = sqp.tile([128, NCH * WD], BF16, tag="gsq", name="gsq")
            gp.tensor_tensor(gsq[:], M_sb[b][:], M_sb[b][:], op=OP.mult)
            dve.tensor_reduce(mq[:], gsq[:].rearrange(
                "q (i w) -> q i w", w=WD), axis=mybir.AxisListType.X,
                op=OP.add)
            msq.append(mq)

        # ---- the Lns, all adjacent in ACT program order ----
        ln_u = sb(BC, 1, "ln_u")
        act.activation(ln_u[:], u_sb[:], AF.Ln)
        wf = sb(BC, 1, "wf")
        act.activation(wf[:], wk2[:], AF.Ln)
        rf = sb(BC, R, "rf")
        act.activation(rf[:], rk2[:], AF.Ln)
        for b in range(BC):
            rw_ = sb(128, NCH, f"rn_w{b}")
            act.activation(rw_[:], msq[b][:], AF.Ln)
            rn_w.append(rw_)

        # ---- exp-table from here on ----
        act.activation(wf[:], wf[:], AF.Exp, scale=-0.5)
        act.activation(rf[:], rf[:], AF.Exp, scale=-0.5)
        for b in range(BC):
            act.activation(rn_w[b][:], rn_w[b][:], AF.Exp, scale=-0.5)
        rm_e = sb(BC, 3 * R, "rm_e")
        act.activation(rm_e[:], v_sb[:, O_RM:O_RM + 3 * R], AF.Exp)
        rm_sum = sb(BC, R, "rm_sum")
        dve.tensor_reduce(rm_sum[:], rm_e[:].rearrange("o (r t) -> o r t", t=3),
                          axis=mybir.AxisListType.X, op=OP.add)
        rm_rec = sb(BC, R, "rm_rec")
        dve.reciprocal(rm_rec[:], rm_sum[:])
        modes = sb(BC, 3 * R, "modes")
        dve.tensor_tensor(modes[:].rearrange("o (r t) -> o r t", t=3),
                          rm_e[:].rearrange("o (r t) -> o r t", t=3),
                          rm_rec[:].rearrange("o (r t) -> o r t", t=1)
                          .broadcast_to([BC, R, 3]),
                          op=OP.mult)

        omu = sb(BC, 1, "omu")
        act.activation(omu[:], u_sb[:], AF.Copy, scale=-1.0, bias=1.0)
        omag = sb(BC, 1, "omag")
        act.activation(omag[:], awg[:, 0:1], AF.Copy, scale=-1.0, bias=1.0)
        c1 = sb(BC, 1, "c1")
        dve.tensor_tensor(c1[:], awg[:, 1:2], awg[:, 0:1], op=OP.mult)
        c2 = sb(BC, 1, "c2")
        dve.tensor_tensor(c2[:], awg[:, 1:2], omag[:], op=OP.mult)
        kn = sb(BC, WD, "kn")
        act.activation(kn[:], v_sb[:, O_WK:O_WK + WD], AF.Copy, scale=wf[:])
        rkn = sb(BC, R * WD, "rkn")
        dve.tensor_tensor(rkn[:].rearrange("o (r w) -> o r w", w=WD),
                          v_sb[:, O_RK:O_RK + R * WD]
                          .rearrange("o (r w) -> o r w", w=WD),
                          rf[:].rearrange("o (r w) -> o r w", w=1)
                          .broadcast_to([BC, R, WD]),
                          op=OP.mult)

        # batched packs, unbatched later via selector matmuls
        sc4 = sb(BC, 4, "sc4")          # [ln_u, 1-u, c1, c2]
        dve.tensor_copy(sc4[:, 0:1], ln_u[:])
        dve.tensor_copy(sc4[:, 1:2], omu[:])
        dve.tensor_copy(sc4[:, 2:3], c1[:])
        dve.tensor_copy(sc4[:, 3:4], c2[:])
        ev2 = sb(BC, 2 * WD, "ev2")     # [erase | write_vector]
        dve.tensor_copy(ev2[:, 0:WD], er_sg[:])
        dve.tensor_copy(ev2[:, WD:2 * WD], v_sb[:, O_WV:O_WV + WD])

        # ====== write content scores for BOTH batches (M-gated, no w dep)
        st = [dict() for _ in range(BC)]
        for b in range(BC):
            s = st[b]
            M3 = M_sb[b][:].rearrange("q (i w) -> q i w", w=WD)
            kn_bc = sb(128, WD, f"kn_bc{b}")
            ptk = ps_small(128, WD)
            mm(ptk[:], sel2[:, 128 * b:128 * (b + 1)], kn[:])
            dve.tensor_copy(kn_bc[:], ptk[:])
            wsc_r = sb(128, NCH, f"wsc_r{b}")
            g64 = scr.tile([128, NCH * WD], BF16, tag=f"g64{b}", name="g64")
            for i in range(NCH):
                dve.scalar_tensor_tensor(
                    out=g64[:, WD * i:WD * (i + 1)], in0=M3[:, i, :],
                    scalar=1.0, in1=kn_bc[:], op0=OP.mult, op1=OP.mult,
                    accum_out=wsc_r[:, i:i + 1])
            s['wsc_r'] = wsc_r
        for b in range(BC):
            s = st[b]
            wsc = sb(128, NCH, f"wsc{b}")
            dve.tensor_tensor(wsc[:], s['wsc_r'][:], rn_w[b][:], op=OP.mult)
            wse = sb(128, NCH, f"wse{b}")
            wse_s = sb(128, 1, f"wse_s{b}")
            act.activation(wse[:], wsc[:], AF.Exp, accum_out=wse_s[:])
            ptt = ps_small(1, 1)
            mm(ptt[:], wse_s[:], ones_col[:])
            totr = sb(1, 1, f"totr{b}")
            dve.reciprocal(totr[:], ptt[:])
            s['wse'], s['totr'] = wse, totr

        # =========== per-batch w chain ===========
        for b in range(BC):
            s = st[b]
            M3 = M_sb[b][:].rearrange("q (i w) -> q i w", w=WD)
            wse, totr = s['wse'], s['totr']

            # [ln_u, 1-u, c1, c2] broadcast to 128 parts; totr separately
            pb4 = ps_small(128, 4)
            mm(pb4[:], sel2[:, 128 * b:128 * (b + 1)], sc4[:])
            scb = sb(128, 4, f"scb{b}")
            dve.tensor_copy(scb[:], pb4[:])
            ptb2 = ps_small(128, 1)
            mm(ptb2[:], ones_row[:], totr[:])
            totb = sb(128, 1, f"totb{b}")
            dve.tensor_copy(totb[:], ptb2[:])

            alle = sb(128, NCH, f"alle{b}")
            act.activation(alle[:], iota[:], AF.Exp, scale=scb[:, 0:1])
            alloc = sb(128, NCH, f"alloc{b}")
            act.activation(alloc[:], alle[:], AF.Copy, scale=scb[:, 1:2])

            cww = sb(128, NCH, f"cww{b}")
            dve.tensor_scalar_mul(cww[:], wse[:], totb[:])
            t2 = sb(128, NCH, f"t2w{b}")
            dve.tensor_scalar_mul(t2[:], cww[:], scb[:, 3:4])
            w_sb = sb(128, NCH, f"w_sb{b}")
            dve.scalar_tensor_tensor(out=w_sb[:], in0=alloc[:],
                                     scalar=scb[:, 2:3], in1=t2[:],
                                     op0=OP.mult, op1=OP.add)
            s['w_sb'] = w_sb

            # stream lhsT: [ones|w] in this batch's column pair, zeros in
            # the other batch's, so both batches share one [4,N] psum group
            oww = bpool.tile([128, 4 * NCH], BF16, tag=f"oww{b}",
                             name="oww")
            oww3 = oww[:].rearrange("q (i t) -> q i t", t=4)
            dve.memset(oww[:], 0.0)
            dve.memset(oww3[:, :, 2 * b], 1.0)
            dve.tensor_copy(oww3[:, :, 2 * b + 1], w_sb[:])
            s['oww3'] = oww3

            wrow = bone.tile([1, N], BF16, tag=f"wrow{b}", name="wrow")
            w_bc = bone.tile([128, N], BF16, tag=f"w_bc{b}", name="w_bc")
            for g in range(4):
                pr = ps_small(1, 512)
                for j in range(4):
                    c = 4 * g + j
                    mm(pr[0:1, 128 * j:128 * (j + 1)], w_sb[:, c:c + 1],
                       i128[:])
                dve.tensor_copy(wrow[0:1, 512 * g:512 * (g + 1)], pr[:])
                pb = ps_small(128, 512)
                mm(pb[:], ones_row_bf[:], wrow[0:1, 512 * g:512 * (g + 1)])
                act.copy(w_bc[:, 512 * g:512 * (g + 1)], pb[:])
            s['w_bc'] = w_bc
            s['wrow'] = wrow

            # W = sum(w), P = sum(p) broadcast [128, 2]
            wsum = sb(1, 1, f"wsum{b}")
            pws = ps_small(1, NCH)
            mm(pws[:], ones_col[:], w_sb[:])
            ws16 = sb(1, NCH, f"ws16{b}")
            dve.tensor_copy(ws16[:], pws[:])
            dve.tensor_reduce(wsum[:], ws16[:], axis=mybir.AxisListType.X,
                              op=OP.add)
            psum_s = sb(1, 1, f"psum_s{b}")
            pps = ps_small(1, NCH)
            mm(pps[:], ones_col[:], pT[b][:])
            ps16 = sb(1, NCH, f"ps16{b}")
            dve.tensor_copy(ps16[:], pps[:])
            dve.tensor_reduce(psum_s[:], ps16[:], axis=mybir.AxisListType.X,
                              op=OP.add)
            pw2 = sb(1, 2, f"pw2{b}")
            dve.tensor_copy(pw2[0:1, 0:1], psum_s[:])
            dve.tensor_copy(pw2[0:1, 1:2], wsum[:])
            pbx = ps_small(128, 2)
            mm(pbx[:], ones_row[:], pw2[:])
            pwb = sb(128, 2, f"pwb{b}")
            dve.tensor_copy(pwb[:], pbx[:])

            # endgame precomputes that need only w and p
            def bcol(col):
                return col.rearrange("q (a o) -> q a o", a=1).broadcast_to(
                    [128, 1, NCH])[:, 0, :]
            omw = sb(128, NCH, f"omw{b}")
            act.activation(omw[:], w_sb[:], AF.Copy, scale=-1.0, bias=1.0)
            r_t1 = sb(128, NCH, f"r_t1{b}")
            gp.tensor_tensor(r_t1[:], bcol(pwb[:, 0:1]), pT[b][:],
                             op=OP.subtract)
            r_t2 = sb(128, NCH, f"r_t2{b}")
            gp.tensor_tensor(r_t2[:], w_sb[:], r_t1[:], op=OP.mult)
            c_t1 = sb(128, NCH, f"c_t1{b}")
            gp.tensor_tensor(c_t1[:], bcol(pwb[:, 1:2]), w_sb[:],
                             op=OP.subtract)
            c_t2 = sb(128, NCH, f"c_t2{b}")
            gp.tensor_tensor(c_t2[:], pT[b][:], c_t1[:], op=OP.mult)
            s['omw'], s['r_t2'], s['c_t2'] = omw, r_t2, c_t2

            # stream accumulator targets
            s['rs0'] = sb(128, NCH, f"rs0{b}")
            s['lw'] = sb(128, NCH, f"lw{b}")

        # ==== memory update + read scores: background tasks interleaved
        # into the stream loop (in-order engines fill per-block slack).
        for b in range(BC):
            s = st[b]
            s['Mn_sb'] = bone.tile([128, NCH * WD], F32, tag=f"Mn{b}",
                                   name="Mn")
            s['Mn3'] = s['Mn_sb'][:].rearrange("q (i w) -> q i w", w=WD)
            s['MnT'] = bone.tile([64, NCH * 128], BF16, tag=f"MnT{b}",
                                 name="MnT")

        def bg_tasks(b):
            s = st[b]
            M3 = M_sb[b][:].rearrange("q (i w) -> q i w", w=WD)
            Mn3 = s['Mn3']
            MnT3 = s['MnT'][:].rearrange("q (i c) -> q i c", c=128)
            w_view = st[b]['w_sb'][:].rearrange(
                "q (i a) -> q i a", a=1).broadcast_to([128, NCH, WD])

            def t_ev():
                # [erase | write_vector] broadcast to all partitions
                pevb = ps_small(128, 2 * WD)
                mm(pevb[:], sel2[:, 128 * b:128 * (b + 1)], ev2[:])
                evb = bpool.tile([128, 2 * WD], F32, tag=f"evb{b}",
                                 name="evb")
                dve.tensor_copy(evb[:], pevb[:])
                s['evb'] = evb
            yield t_ev

            def t_mn(step):
                # Mn = M - M*(w x e) + (w x v), all SBUF elementwise
                e_view = s['evb'][:, 0:WD].rearrange(
                    "q (a w) -> q a w", a=1).broadcast_to([128, NCH, WD])
                v_view = s['evb'][:, WD:2 * WD].rearrange(
                    "q (a w) -> q a w", a=1).broadcast_to([128, NCH, WD])
                if step == 0:
                    P = bone.tile([128, NCH * WD], BF16, tag=f"P{b}",
                                  name="P")
                    gp.tensor_tensor(
                        P[:].rearrange("q (i w) -> q i w", w=WD),
                        w_view, e_view, op=OP.mult)
                    s['P'] = P
                elif step == 1:
                    G = bone.tile([128, NCH * WD], BF16, tag=f"G{b}",
                                  name="G")
                    gp.tensor_tensor(
                        G[:].rearrange("q (i w) -> q i w", w=WD),
                        w_view, v_view, op=OP.mult)
                    s['G'] = G
                elif step == 2:
                    t1 = sqp.tile([128, NCH * WD], BF16, tag="gsq",
                                  name="gsq")
                    gp.tensor_tensor(t1[:], M_sb[b][:], s['P'][:],
                                     op=OP.mult)
                    s['t1'] = t1
                elif step == 3:
                    gp.tensor_tensor(s['Mn_sb'][:], M_sb[b][:],
                                     s['t1'][:], op=OP.subtract)
                else:
                    gp.tensor_tensor(s['Mn_sb'][:], s['Mn_sb'][:],
                                     s['G'][:], op=OP.add)
            for step_ in range(5):
                yield (lambda step_=step_: t_mn(step_))

            def t_mq2(g):
                # squared row norms of Mn: Pool product, DVE reduce
                if g == 0:
                    s['gs2'] = sqp.tile([128, NCH * WD], BF16, tag="gsq",
                                        name="gsq")
                    gp.tensor_tensor(s['gs2'][:], s['Mn_sb'][:],
                                     s['Mn_sb'][:], op=OP.mult)
                else:
                    s['mq2'] = sb(128, NCH, f"mq2{b}")
                    dve.tensor_reduce(s['mq2'][:], s['gs2'][:].rearrange(
                        "q (i w) -> q i w", w=WD),
                        axis=mybir.AxisListType.X, op=OP.add)
            for g in range(2):
                yield (lambda g=g: t_mq2(g))

            def t_rn2_ln():
                rn2 = sb(128, NCH, f"rn2{b}")
                act.activation(rn2[:], s['mq2'][:], AF.Ln)
                s['rn2'] = rn2
            yield t_rn2_ln

            def t_rn2_exp():
                act.activation(s['rn2'][:], s['rn2'][:], AF.Exp, scale=-0.5)
            yield t_rn2_exp

            def t_mnt(g):
                ptm = ps_small(64, 512)
                for j in range(4):
                    pe.transpose(ptm[:, 128 * j:128 * (j + 1)],
                                 Mn3[:, 4 * g + j, :], i128[:])
                act.copy(s['MnT'][0:64, 512 * g:512 * (g + 1)], ptm[:])
            for g in range(4):
                yield (lambda g=g: t_mnt(g))

            def t_rknt():
                rknp = ps_small(1, R * WD)
                mm(rknp[:], i128[0:BC, b:b + 1], rkn[:])
                rkb = sb(1, R * WD, f"rkb{b}")
                dve.tensor_copy(rkb[:], rknp[:])
                rknT = bpool.tile([64, R], BF16, tag=f"rknT{b}",
                                  name="rknT")
                ptk2 = ps_small(64, R)
                for r in range(R):
                    mm(ptk2[:, r:r + 1],
                       rkb[0:1, WD * r:WD * (r + 1)],
                       one_f32[0:1, 0:1])
                dve.tensor_copy(rknT[:], ptk2[:])
                s['rknT'] = rknT
                s['rsc'] = sb(128, R * NCH, f"rsc{b}")
            yield t_rknt

            def t_rsc(g):
                rsc3 = s['rsc'][:].rearrange("q (r i) -> q r i", i=NCH)
                for i in range(4 * g, 4 * g + 4):
                    ptr = ps_small(128, R)
                    mm(ptr[:], MnT3[:, i, :], s['rknT'][:])
                    dve.tensor_scalar_mul(rsc3[:, :, i], ptr[:],
                                          s['rn2'][:, i:i + 1])
            for g in range(4):
                yield (lambda g=g: t_rsc(g))

            def t_rex():
                rsc3 = s['rsc'][:].rearrange("q (r i) -> q r i", i=NCH)
                rex = sb(128, R * NCH, f"rex{b}")
                rex3 = rex[:].rearrange("q (r i) -> q r i", i=NCH)
                res_s = sb(128, R, f"res_s{b}")
                for r in range(R):
                    act.activation(rex3[:, r, :], rsc3[:, r, :], AF.Exp,
                                   accum_out=res_s[:, r:r + 1])
                ptot = ps_small(R, 1)
                mm(ptot[:], res_s[:], ones_col[:])
                rec4 = sb(R, 1, f"rec4{b}")
                dve.reciprocal(rec4[:], ptot[:])
                prr = ps_small(1, R)
                mm(prr[:], rec4[:], i128[0:R, 0:R])
                rec_row = sb(1, R, f"rec_row{b}")
                dve.tensor_copy(rec_row[:], prr[:])
                s['rex3'] = rex3
                s['rec_row'] = rec_row
            yield t_rex

        tasks = []
        gens = [bg_tasks(b) for b in range(BC)]
        alive = [True, True]
        while any(alive):
            for b in range(BC):
                if alive[b]:
                    try:
                        tasks.append(next(gens[b]))
                    except StopIteration:
                        alive[b] = False

        # =========== the L stream: both batches interleaved ===========
        cscw_ps = pbig.tile([4, N], F32, tag="cscw", name="cscw")
        ntask = len(tasks)
        done = 0
        for i in range(NCH):
            for b in range(BC):
                s = st[b]
                lblk = lpool.tile([128, N], F32, tag="lblk", name="lblk")
                nc.sync.dma_start(lblk[:], l_ap[b, 128 * i:128 * (i + 1), :])
                lb = lbf.tile([128, N], BF16, tag="lbf", name="lbf")
                act.activation(lb[:], lblk[:], AF.Copy,
                               accum_out=s['rs0'][:, i:i + 1])
                for c in range(4):
                    mm(cscw_ps[:, 512 * c:512 * (c + 1)],
                       s['oww3'][:, i, :], lb[:, 512 * c:512 * (c + 1)],
                       start=(i == 0 and b == 0),
                       stop=(i == NCH - 1 and b == BC - 1))
                sT = scr.tile([128, N], BF16, tag="sttr", name="sttr")
                dve.scalar_tensor_tensor(
                    out=sT[:], in0=lblk[:], scalar=1.0, in1=s['w_bc'][:],
                    op0=OP.mult, op1=OP.mult,
                    accum_out=s['lw'][:, i:i + 1])
            want = 0 if i < 4 else (i - 3) * ntask // (NCH - 4)
            while done < want:
                tasks[done]()
                done += 1

        # =========== endgame ===========
        # bwd chains first (independent of the colsum readout)
        for b in range(BC):
            s = st[b]
            rr1 = sb(128, NCH, f"rr1{b}")
            gp.tensor_tensor(rr1[:], s['omw'][:], s['rs0'][:], op=OP.mult)
            gp.tensor_tensor(rr1[:], rr1[:], s['lw'][:], op=OP.subtract)
            gp.tensor_tensor(rr1[:], rr1[:], s['r_t2'][:], op=OP.add)
            ebw = sb(128, NCH, f"ebw{b}")
            ebw_s = sb(128, 1, f"ebw_s{b}")
            act.activation(ebw[:], rr1[:], AF.Exp, scale=1.0 / N,
                           accum_out=ebw_s[:])
            s['ebw'], s['ebw_s'] = ebw, ebw_s

        # shared colsum readout, pipelined in 512-col chunks
        cscw_sb = bone.tile([4, N], F32, tag="cscw_sb", name="cscw_sb")
        csT = bone.tile([128, 4 * NCH], F32, tag="csT", name="csT")
        csT3 = csT[:].rearrange("q (i t) -> q i t", t=4)
        ptc = ps_small(128, 4 * NCH)
        for g in range(4):
            act.copy(cscw_sb[:, 512 * g:512 * (g + 1)],
                     cscw_ps[:, 512 * g:512 * (g + 1)])
            for c in range(4 * g, 4 * g + 4):
                mm(ptc[:, 4 * c:4 * c + 4],
                   cscw_sb[0:4, 128 * c:128 * (c + 1)], i128[0:4, 0:4])
        dve.tensor_copy(csT[:], ptc[:])

        # fwd chains
        for b in range(BC):
            s = st[b]
            cc1 = sb(128, NCH, f"cc1{b}")
            gp.tensor_tensor(cc1[:], s['omw'][:], csT3[:, :, 2 * b],
                             op=OP.mult)
            gp.tensor_tensor(cc1[:], cc1[:], csT3[:, :, 2 * b + 1],
                             op=OP.subtract)
            gp.tensor_tensor(cc1[:], cc1[:], s['c_t2'][:], op=OP.add)
            efw = sb(128, NCH, f"efw{b}")
            efw_s = sb(128, 1, f"efw_s{b}")
            act.activation(efw[:], cc1[:], AF.Exp, scale=1.0 / N,
                           accum_out=efw_s[:])
            s['efw'], s['efw_s'] = efw, efw_s

        # normalizer-folded head coefficients
        for b in range(BC):
            s = st[b]
            ptb = ps_small(1, 2)
            mm(ptb[0:1, 0:1], s['ebw_s'][:], ones_col[:])
            mm(ptb[0:1, 1:2], s['efw_s'][:], ones_col[:])
            rec_bf = sb(1, 2, f"rec_bf{b}")
            dve.reciprocal(rec_bf[:], ptb[:])
            mptr = ps_small(1, 3 * R)
            mm(mptr[:], i128[0:BC, b:b + 1], modes[:])
            mo_b = sb(1, 3 * R, f"mo_b{b}")
            dve.tensor_copy(mo_b[:], mptr[:])
            bvec = sb(1, 3 * R, f"bvec{b}")
            m3v = mo_b[:].rearrange("o (r t) -> o r t", t=3)
            dve.tensor_tensor(bvec[0:1, 0:R], m3v[:, :, 0],
                              rec_bf[0:1, 0:1].broadcast_to([1, R]),
                              op=OP.mult)
            dve.tensor_tensor(bvec[0:1, R:2 * R], m3v[:, :, 1],
                              s['rec_row'][:], op=OP.mult)
            dve.tensor_tensor(bvec[0:1, 2 * R:3 * R], m3v[:, :, 2],
                              rec_bf[0:1, 1:2].broadcast_to([1, R]),
                              op=OP.mult)
            pbv = ps_small(128, 3 * R)
            mm(pbv[:], ones_row[:], bvec[:])
            Bco = sb(128, 3 * R, f"Bco{b}")
            dve.tensor_copy(Bco[:], pbv[:])
            s['B3'] = Bco[:].rearrange("q (t r) -> q t r", r=R)

        # read weights on Pool: rw = B0_r*ebw + B1_r*rex + B2_r*efw
        for b in range(BC):
            s = st[b]
            B3 = s['B3']
            rw_sb = sb(128, R * NCH, f"rw_sb{b}")
            rw3 = rw_sb[:].rearrange("q (r i) -> q r i", i=NCH)
            ebw_b = s['ebw'][:].rearrange("q (a i) -> q a i", a=1)\
                .broadcast_to([128, R, NCH])
            efw_b = s['efw'][:].rearrange("q (a i) -> q a i", a=1)\
                .broadcast_to([128, R, NCH])
            z1 = sb(128, R * NCH, f"z1{b}")
            z13 = z1[:].rearrange("q (r i) -> q r i", i=NCH)
            gp.tensor_tensor(
                rw3[:], ebw_b,
                B3[:, 0, :].rearrange("q (r a) -> q r a", a=1)
                .broadcast_to([128, R, NCH]), op=OP.mult)
            gp.tensor_tensor(
                z13[:], s['rex3'][:],
                B3[:, 1, :].rearrange("q (r a) -> q r a", a=1)
                .broadcast_to([128, R, NCH]), op=OP.mult)
            gp.tensor_tensor(rw3[:], rw3[:], z13[:], op=OP.add)
            gp.tensor_tensor(
                z13[:], efw_b,
                B3[:, 2, :].rearrange("q (r a) -> q r a", a=1)
                .broadcast_to([128, R, NCH]), op=OP.mult)
            gp.tensor_tensor(rw3[:], rw3[:], z13[:], op=OP.add)
            s['rw_by_i'] = rw_sb[:].rearrange("q (r i) -> q i r", i=NCH)

        # read vectors: both batches' psum chains interleaved on PE
        prv = [pacc.tile([R, WD], F32, tag="pacc", name="pacc")
               for _ in range(BC)]
        for i in range(NCH):
            for b in range(BC):
                mm(prv[b][:], st[b]['rw_by_i'][:, i, :],
                   st[b]['Mn3'][:, i, :],
                   start=(i == 0), stop=(i == NCH - 1))
        for b in range(BC):
            out_sb = sb(R, WD, f"out_sb{b}")
            dve.tensor_copy(out_sb[:], prv[b][:])
            nc.sync.dma_start(out_ap[b], out_sb[:])

    nc.compile()
    return nc


_NC_CACHE = []


def kernel(x, memory, L, p, W1, b1, W2, b2):
    import ml_dtypes
    BF = ml_dtypes.bfloat16
    x = np.ascontiguousarray(x, np.float32).astype(BF)
    memory = np.ascontiguousarray(memory, np.float32)
    L = np.ascontiguousarray(L, np.float32)
    p = np.ascontiguousarray(p, np.float32)
    W1 = np.ascontiguousarray(W1, np.float32).astype(BF)
    b1 = np.ascontiguousarray(b1, np.float32).reshape(1, H_D).astype(BF)
    W2 = np.ascontiguousarray(W2, np.float32).astype(BF)
    b2 = np.ascontiguousarray(b2, np.float32).reshape(1, IFACE).astype(BF)

    iota = (np.arange(N, dtype=np.float32).reshape(NCH, 128).T + 1.0).copy()
    i128 = np.eye(128, dtype=np.float32)
    sel2 = np.zeros((BC, BC * 128), dtype=np.float32)
    for b in range(BC):
        sel2[b, 128 * b:128 * (b + 1)] = 1.0

    if not _NC_CACHE:
        _NC_CACHE.append(build_nc())
    nc = _NC_CACHE[0]

    in_maps = []
    for c in range(NCORES):
        s = slice(BC * c, BC * (c + 1))
        in_maps.append({
            'x': x[s], 'memory': memory[s], 'L': L[s], 'p': p[s],
            'W1': W1, 'b1': b1, 'W2': W2, 'b2': b2,
            'iota_p1': iota, 'i128': i128, 'sel2': sel2,
        })

    res = run_bass_kernel_spmd(nc, in_maps, list(range(NCORES)))
    outs = [res.results[c]['out'].reshape(BC, 1, R * WD)
            for c in range(NCORES)]
    return np.concatenate(outs, axis=0)
